# revision 1
# baseline (speedup 1.0000x reference)
"""Multi-head self-attention (16 heads, hd=64, RoPE, causal) on 8 trn2 cores.

Sharding: DP(batch=2) x TP(head-groups=4). Core c handles batch c//4, heads
[4*(c%4), 4*(c%4)+4). Each core computes a row-parallel partial output
yT_partial [1024, 2048]; host sums the 4 partials per batch and transposes.
No device-device communication.

Device kernel (v2, software-pipelined):
  - bf16 x / wqkv / wo / q / k / v / trig / exp-weights / normalized
    attention; fp32 PSUM accumulation and fp32 output partials.
  - transposed layout throughout: xT [e,t], qT/kT [128, t] per head-pair
    (per-head rows de-interleaved [evens|odds] so the RoPE partner swap is
    row^32), scoresT [kt, q] per head, attnT via v_aug ones-column trick.
  - RoPE partner swap computed on PE with a 0/1 permutation matmul
    (no partition-swap DMAs).
  - one exp activation per kt step covering both heads of a pair (3D AP over
    a [128, 1024] PSUM tile).
  - attention emitted as q-windows of 512 cols; a filler FIFO interleaves
    projection / output-projection matmuls between attention steps so the
    PE never idles (the cost model halves PE speed for 3us after any idle).
  - warmup matmuls on a zeroed tile bridge the initial DMA wait.
"""

import sys

for _p in ("/opt/trn_rl_repo",):
    if _p not in sys.path:
        sys.path.insert(0, _p)

from collections import deque
from contextlib import ExitStack

import numpy as np

import concourse.bass as bass
import concourse.mybir as mybir
import concourse.tile as tile
from concourse import bacc
from concourse.bass_utils import run_bass_kernel_spmd

F32 = mybir.dt.float32
F32R = mybir.dt.float32r
BF16 = mybir.dt.bfloat16
AF = mybir.ActivationFunctionType

B, T, E = 2, 2048, 1024
NH, HD = 16, 64
NHL = 4          # heads per core
DL = NHL * HD    # 256 local head dims
NCORES = 8
NEG = -1e9
ROPE_BASE = 10000.0

QW = 512         # attention q-window
NWIN = T // QW   # 4 windows
NBLK = 4         # projection t-blocks of 512
N_WARM = 42      # warmup matmuls (N=256) bridging the initial DMA wait
N_BRIDGE = 42    # tail-bridge matmuls through the last norm/DMA latency


# ----------------------------------------------------------------- device IR
def build_module(reps=1):
    nc = bacc.Bacc("TRN2", target_bir_lowering=False, debug=False,
                   num_devices=NCORES)

    xt = nc.dram_tensor("xt", [E, T], BF16, kind="ExternalInput").ap()
    wqkv = nc.dram_tensor("wqkv", [E, 3 * DL], BF16, kind="ExternalInput").ap()
    wot = nc.dram_tensor("wot", [DL, E], BF16, kind="ExternalInput").ap()
    trig = nc.dram_tensor("trig", [2, 128, T], BF16, kind="ExternalInput").ap()
    consts = nc.dram_tensor("consts", [128, 384], BF16,
                            kind="ExternalInput").ap()
    yt = nc.dram_tensor("yt", [E, T], F32, kind="ExternalOutput").ap()

    with tile.TileContext(nc) as tc:
        for _ in range(reps):
            _body(tc, xt, wqkv, wot, trig, consts, yt)
    nc.compile()
    return nc


def _body(tc, xt, wqkv, wot, trig, consts, yt):
    nc = tc.nc

    with ExitStack() as ctx:
        po = ctx.enter_context(tc.tile_pool(name="po", bufs=1))
        xcp = ctx.enter_context(tc.tile_pool(name="xcp", bufs=16))
        rp = ctx.enter_context(tc.tile_pool(name="rp", bufs=5))
        ep = ctx.enter_context(tc.tile_pool(name="ep", bufs=4))
        dp = ctx.enter_context(tc.tile_pool(name="dp", bufs=4))
        yp = ctx.enter_context(tc.tile_pool(name="yp", bufs=2))
        pjp = ctx.enter_context(tc.tile_pool(name="pjp", bufs=2, space="PSUM"))
        ssp = ctx.enter_context(tc.tile_pool(name="ssp", bufs=2, space="PSUM"))
        sap = ctx.enter_context(tc.tile_pool(name="sap", bufs=2, space="PSUM"))

        # ---------------- persistent tiles --------------------------------
        # qk[0]=q pair0, qk[1]=q pair1, qk[2]=k pair0, qk[3]=k pair1
        qk = [po.tile([128, T], BF16, tag=f"qk{i}", name=f"qk{i}")
              for i in range(4)]
        v_sb = po.tile([128, 16 * 260], BF16, tag="v", name="v_sb")
        w_sb = po.tile([128, 8 * 768], BF16, tag="w", name="w_sb")
        wot_sb = [po.tile([128, E], BF16, tag=f"wot{p}", name=f"wot{p}")
                  for p in range(2)]
        trigc = po.tile([128, T], BF16, tag="tc", name="trigc")
        trigs = po.tile([128, T], BF16, tag="tsn", name="trigs")
        cst = po.tile([128, 384], BF16, tag="cst", name="cst")
        negi, stepm, permm = cst[:, 0:128], cst[:, 128:256], cst[:, 256:384]
        at = [po.tile([128, T], BF16, tag=f"at{p}", name=f"at{p}")
              for p in range(2)]
        warm = po.tile([128, 384], BF16, tag="warm", name="warm")

        ones_sb = po.tile([65, 64], F32, tag="ones", name="ones_sb")

        # ---------------- init: memsets + DMAs -----------------------------
        nc.gpsimd.memset(warm[:], 0.0)
        nc.gpsimd.memset(ones_sb[64:65, 0:64], 1.0)
        v_ones = v_sb[:].rearrange("p (kt h x) -> p kt h x", kt=16, h=4)
        nc.gpsimd.memset(v_ones[:, :, :, 64:65], 1.0)

        w_v = w_sb[:].rearrange("p (eo d) -> p eo d", eo=8)
        wqkv_v = wqkv.rearrange("(eo p) d -> p eo d", p=128)

        xc = {}

        def load_x(b, eos=range(8)):
            for eo in eos:
                t_ = xcp.tile([128, 512], BF16, tag="xc", name="xc")
                nc.sync.dma_start(
                    out=t_[:],
                    in_=xt[eo * 128:(eo + 1) * 128,
                           b * 512:(b + 1) * 512])
                xc[(b, eo)] = t_

        # issue order tuned so each transfer lands just before first use;
        # wq split by eo-half so the first projection group starts early
        nc.sync.dma_start(out=w_v[:, 0:4, 0:256], in_=wqkv_v[:, 0:4, 0:256])
        load_x(0, range(0, 4))
        nc.sync.dma_start(out=w_v[:, 4:8, 0:256], in_=wqkv_v[:, 4:8, 0:256])
        load_x(0, range(4, 8))
        nc.sync.dma_start(out=w_v[:, :, 256:512], in_=wqkv_v[:, :, 256:512])
        nc.sync.dma_start(out=w_v[:, :, 512:768], in_=wqkv_v[:, :, 512:768])
        nc.sync.dma_start(out=trigc[:], in_=trig[0])
        nc.sync.dma_start(out=trigs[:], in_=trig[1])
        load_x(1)
        nc.sync.dma_start(out=cst[:], in_=consts[:])
        load_x(2)
        load_x(3)
        for p in range(2):
            nc.sync.dma_start(out=wot_sb[p][:],
                              in_=wot[p * 128:(p + 1) * 128, :])

        # activation-table load lands during the DMA wait
        nc.scalar.activation(warm[0:1, 256:257], warm[0:1, 0:1], AF.Exp)

        # warmup: keep PE busy (and ramping) until the first x chunks land
        warm_ps = pjp.tile([128, 256], F32, tag="pj", name="warm_ps")
        for i in range(N_WARM):
            nc.tensor.matmul(out=warm_ps[:], lhsT=warm[:, 0:128],
                             rhs=warm[:, 128:384],
                             start=(i == 0), stop=(i == N_WARM - 1))

        # ---------------- projection + rope emission helpers ---------------
        # nm: 0=q0, 1=q1, 2=k0, 3=k1 ; block b covers t cols [512b, 512b+512)
        def qk_mms(nm, b):
            """8 accumulating MMs + psum->bf16 copy; returns raw/psum tiles."""
            wcol = (nm % 2) * 128 if nm < 2 else 256 + (nm % 2) * 128
            ps = pjp.tile([128, 512], F32, tag="pj", name="pjqk")
            for eo in range(8):
                nc.tensor.matmul(
                    out=ps[:],
                    lhsT=w_sb[:, eo * 768 + wcol: eo * 768 + wcol + 128],
                    rhs=xc[(b, eo)][:],
                    start=(eo == 0), stop=(eo == 7))
            raw = rp.tile([128, 512], BF16, tag="raw", name="raw")
            if b == 0 and nm < 2:
                nc.scalar.copy(raw[:], ps[:])   # ACT is idle pre-attention
            else:
                nc.vector.tensor_copy(raw[:], ps[:])
            return raw

        def qk_rope(nm, b, raw):
            """perm matmul + cos/sin muls + add into qk[nm] block cols."""
            cs = slice(b * 512, b * 512 + 512)
            swp = pjp.tile([128, 512], F32, tag="pj", name="pjswp")
            nc.tensor.matmul(out=swp[:], lhsT=permm, rhs=raw[:],
                             start=True, stop=True)
            nc.gpsimd.tensor_mul(qk[nm][:, cs], raw[:], trigc[:, cs])
            tmp = rp.tile([128, 512], BF16, tag="tmp", name="tmp")
            nc.vector.tensor_mul(tmp[:], swp[:], trigs[:, cs])
            nc.gpsimd.tensor_add(qk[nm][:, cs], qk[nm][:, cs], tmp[:])

        def v_grp(b, tt):
            """one 128-t-row V projection group; kt block = 4b+tt."""
            ps = pjp.tile([128, 256], F32, tag="pj", name="pjv")
            for eo in range(8):
                nc.tensor.matmul(
                    out=ps[:],
                    lhsT=xc[(b, eo)][:, tt * 128:tt * 128 + 128],
                    rhs=w_v[:, eo, 512:768],
                    start=(eo == 0), stop=(eo == 7))
            kt = 4 * b + tt
            dst = v_sb[:, kt * 260:(kt + 1) * 260] \
                .rearrange("p (h x) -> p h x", h=4)
            if b == 0 and tt % 2 == 0:
                nc.scalar.copy(dst[:, :, 0:64],
                               ps[:].rearrange("p (h x) -> p h x", h=4))
            else:
                nc.vector.tensor_copy(dst[:, :, 0:64],
                                      ps[:].rearrange("p (h x) -> p h x", h=4))

        # ---------------- filler FIFO --------------------------------------
        fifo = deque()
        emitted = set()
        # rough PE-time of each item kind, for the debt-based pump
        COSTS = {"qkA": 1700.0, "qkB": 260.0, "v": 900.0, "yt": 480.0}
        debt = [0.0]

        def enqueue_block(b, b0_order=False):
            raws = {}
            if b0_order:
                # DMA arrival order at startup: A's first, then v/B
                # interleaved (psum-ring WARs hide behind alternation)
                order = [("A", 0), ("A", 2), ("A", 1), ("A", 3),
                         ("v", 0), ("B", 0), ("v", 1), ("B", 2),
                         ("v", 2), ("B", 1), ("v", 3), ("B", 3)]
            else:
                order = [("A", 0), ("A", 2), ("B", 0), ("A", 1), ("B", 2),
                         ("A", 3), ("B", 1), ("B", 3),
                         ("v", 0), ("v", 1), ("v", 2), ("v", 3)]
            for kind, x in order:
                if kind == "A":
                    fifo.append((("qkA", x, b),
                                 lambda nm=x, b=b: raws.__setitem__(
                                     nm, qk_mms(nm, b))))
                elif kind == "B":
                    fifo.append((("qkB", x, b),
                                 lambda nm=x, b=b: qk_rope(nm, b,
                                                           raws.pop(nm))))
                else:
                    fifo.append((("v", b, x),
                                 lambda b=b, tt=x: v_grp(b, tt)))

        def emit_next():
            tag, fn = fifo.popleft()
            fn()
            emitted.add(tag)

        reserve = [0]

        def pump_ns(ns):
            debt[0] += ns
            while len(fifo) > reserve[0] and debt[0] >= COSTS[fifo[0][0][0]]:
                k = fifo[0][0][0]
                emit_next()
                debt[0] -= COSTS[k]

        def force(tag):
            if tag in emitted:
                return
            while fifo:
                t, _ = fifo[0]
                emit_next()
                if t == tag:
                    debt[0] = 0.0
                    return
            raise AssertionError(f"force: {tag} never enqueued")

        # ---------------- attention ----------------------------------------
        def att_call(P, W):
            """attention for pair P, q cols [512W, 512W+512)."""
            nkt = 4 * W + 4
            qcols = slice(W * 512, W * 512 + 512)
            # rope of q[P] block W and k[P] blocks <= W must be emitted
            force(("qkB", P, W))
            for bb in range(W + 1):
                force(("qkB", 2 + P, bb))

            ps_a = [sap.tile([128, 512], F32, tag="a", name="ps_a")
                    for _ in range(2)]
            exps = [None] * nkt

            def scores_step(kt):
                qs = max(0, 128 * kt - 512 * W)
                diag = kt >= 4 * W
                ss = ssp.tile([128, 1024], F32, tag="s", name="ss")
                for h in range(2):
                    nc.tensor.matmul(
                        out=ss[:, h * 512 + qs: h * 512 + 512],
                        lhsT=qk[2 + P][h * 64:h * 64 + 64,
                                       kt * 128:kt * 128 + 128],
                        rhs=qk[P][h * 64:h * 64 + 64, W * 512 + qs:
                                  W * 512 + 512],
                        start=True, stop=not diag,
                        tile_position=(h * 64, 0))
                if diag:
                    for h in range(2):
                        nc.tensor.matmul(
                            out=ss[:, h * 512 + qs: h * 512 + qs + 128],
                            lhsT=negi, rhs=stepm,
                            start=False, stop=True)
                e = ep.tile([128, 1024], BF16, tag="e", name="exp_t")
                e3 = e[:].rearrange("p (h c) -> p h c", h=2)[:, :, qs:512]
                s3 = ss[:].rearrange("p (h c) -> p h c", h=2)[:, :, qs:512]
                nc.scalar.activation(e3, s3, AF.Exp)
                exps[kt] = (e, qs)

            def attnv_step(kt):
                e, qs = exps[kt]
                for h in range(2):
                    slot = kt * 260 + (2 * P + h) * 65
                    nc.tensor.matmul(
                        out=ps_a[h][0:65, qs:512],
                        lhsT=v_sb[:, slot:slot + 65],
                        rhs=e[:, h * 512 + qs: h * 512 + 512],
                        start=(kt == 0), stop=(kt == nkt - 1))
                exps[kt] = None

            for step in range(nkt + 1):
                if step < nkt:
                    # pre-force v blocks one block ahead of the kt cursor
                    vb = min(step // 4 + 1, W)
                    for bb in range(vb + 1):
                        for tt in range(4):
                            if (("v", bb, tt)) not in emitted:
                                force(("v", bb, tt))
                    scores_step(step)
                if step > 0:
                    attnv_step(step - 1)
                # ACT-vs-PE imbalance this step, paid to the filler pump
                qs = max(0, 128 * min(step, nkt - 1) - 512 * W)
                cols = 512 - qs
                gap = (2 * cols * 0.833 + 500.0) - (4 * cols * 0.4167 + 107.0)
                pump_ns(max(200.0, gap))

            # ---------------- normalization -------------------------------
            # denominators: pbcast psum row 64 -> [64,512], recip, then mul.
            # h1 first: its a1n staging DMA is the longest pole into yt.
            last_call = (P == 1 and W == NWIN - 1)
            pump_ns(600.0)
            # stage denom rows to SBUF (ACT), then K=1 broadcast matmul
            dh = [dp.tile([65, 512], F32R, tag="dh", name="dh")
                  for _ in range(2)]
            nc.scalar.copy(dh[1][64:65, :], ps_a[1][64:65, :])
            nc.vector.tensor_copy(dh[0][64:65, :], ps_a[0][64:65, :])
            ps_b = [None, None]
            for h in (1, 0):
                ps_b[h] = pjp.tile([128, 512], F32, tag="pj", name="ps_b")
                nc.tensor.matmul(out=ps_b[h][0:64, :],
                                 lhsT=ones_sb[64:65, 0:64].bitcast(F32R),
                                 rhs=dh[h][64:65, :],
                                 start=True, stop=True,
                                 tile_position=(64, 0))
            if last_call:
                # dependency-free bridge: keep PE busy (and un-throttled)
                # through the recip + mul + a1n-DMA latency before yt
                bridge_ps = pjp.tile([128, 256], F32, tag="pj",
                                     name="bridge_ps")
                for i in range(N_BRIDGE):
                    nc.tensor.matmul(out=bridge_ps[:], lhsT=warm[:, 0:128],
                                     rhs=warm[:, 128:384],
                                     start=(i == 0), stop=(i == N_BRIDGE - 1))
            rc = [dp.tile([64, 512], F32, tag="rc", name="rc")
                  for _ in range(2)]
            nc.vector.reciprocal_approx_fast(out=rc[1][0:64, :],
                                             in_=ps_b[1][0:64, :])
            pump_ns(600.0)
            a1n = dp.tile([64, 512], BF16, tag="a1n", name="a1n")
            nc.vector.tensor_mul(a1n[0:64, :], ps_a[1][0:64, :],
                                 rc[1][0:64, :])
            nc.sync.dma_start(out=at[P][64:128, qcols], in_=a1n[0:64, :])
            nc.vector.reciprocal_approx_fast(out=rc[0][0:64, :],
                                             in_=ps_b[0][0:64, :])
            pump_ns(600.0)
            nc.vector.tensor_mul(at[P][0:64, qcols], ps_a[0][0:64, :],
                                 rc[0][0:64, :])

        # ---------------- output projection --------------------------------
        ytv = yt.rearrange("(et p) t -> p et t", p=128)

        def enqueue_yt(W):
            qcols = slice(W * 512, W * 512 + 512)
            y_sb = yp.tile([128, 8 * 512], F32, tag="ysb", name="y_sb")

            nst = 1 if W == NWIN - 1 else 4   # store granularity (ets)

            def yt_grp(et):
                ps_y = pjp.tile([128, 512], F32, tag="pj", name="ps_y")
                for p in range(2):
                    nc.tensor.matmul(
                        out=ps_y[:],
                        lhsT=wot_sb[p][:, et * 128:(et + 1) * 128],
                        rhs=at[p][:, qcols],
                        start=(p == 0), stop=(p == 1))
                ydst = y_sb[:, et * 512:(et + 1) * 512]
                if W == NWIN - 1 and et % 2 == 0:
                    # exp stream is finished: ACT takes half the tail copies
                    nc.scalar.copy(ydst, ps_y[:])
                else:
                    nc.vector.tensor_copy(ydst, ps_y[:])
                if et % nst == nst - 1:
                    eg = et // nst
                    nc.sync.dma_start(
                        out=ytv[:, eg * nst:(eg + 1) * nst, qcols],
                        in_=y_sb[:, eg * nst * 512:(eg + 1) * nst * 512]
                        .rearrange("p (et t) -> p et t", et=nst))

            for et in range(8):
                fifo.append((("yt", W, et), lambda et=et: yt_grp(et)))

        # ---------------- master schedule -----------------------------------
        # block 0 emitted straight; blocks 1..3 via the FIFO
        enqueue_block(0, b0_order=True)
        while fifo:
            emit_next()
        for b in range(1, NBLK):
            enqueue_block(b)

        RESV = {(0, 2): 8, (1, 2): 8, (0, 3): 8, (1, 3): 0}
        for W in range(NWIN):
            for P in range(2):
                reserve[0] = RESV.get((P, W), 0)
                att_call(P, W)
            enqueue_yt(W)
        while fifo:
            emit_next()


# ----------------------------------------------------------------- host side
def _prep_core_inputs(x, wq, wk, wv, wo):
    """Build the 8 per-core input dicts."""
    import ml_dtypes
    bf = ml_dtypes.bfloat16

    inv_freq = 1.0 / (ROPE_BASE ** (np.arange(0, HD, 2, dtype=np.float32) / HD))
    pos = np.arange(T, dtype=np.float32)
    freqs = pos[:, None] * inv_freq[None, :]          # [T, 32]
    cosT = np.cos(freqs).T.astype(np.float32)          # [32, T]
    sinT = np.sin(freqs).T.astype(np.float32)
    C = np.tile(cosT, (4, 1))                          # [128, T]
    S = np.tile(np.concatenate([-sinT, sinT], axis=0), (2, 1))
    trig = np.stack([C, S]).astype(bf)                 # [2, 128, T]
    scale = np.float32(1.0 / np.sqrt(HD))              # folded into wq

    negid = (np.eye(128, dtype=np.float32) * NEG)
    r, c = np.indices((128, 128))
    stepd = (c < r).astype(np.float32)
    permM = (r == (c ^ 32)).astype(np.float32)         # lhsT[p,r]=1 iff p=r^32
    consts = np.concatenate([negid, stepd, permM], axis=1).astype(bf)

    perm64 = np.concatenate([np.arange(0, HD, 2), np.arange(1, HD, 2)])

    xts = [np.ascontiguousarray(x[b_].T).astype(bf) for b_ in range(B)]
    in_maps = []
    for core in range(NCORES):
        b_, hg = divmod(core, 4)
        heads = np.arange(4 * hg, 4 * hg + 4)
        qk_rows = np.concatenate([h * HD + perm64 for h in heads])
        v_rows = np.concatenate([h * HD + np.arange(HD) for h in heads])
        wq_t = wq.T[:, qk_rows] * scale                # [E, 256]
        wk_t = wk.T[:, qk_rows]
        wv_t = wv.T[:, v_rows]
        wqkv = np.concatenate([wq_t, wk_t, wv_t], axis=1).astype(bf)
        wot_ = np.ascontiguousarray(wo.T[v_rows, :]).astype(bf)
        in_maps.append({
            "xt": xts[b_], "wqkv": wqkv, "wot": wot_,
            "trig": trig, "consts": consts,
        })
    return in_maps


_NC_CACHE = {}


def _get_module():
    if "nc" not in _NC_CACHE:
        _NC_CACHE["nc"] = build_module()
    return _NC_CACHE["nc"]


def _get_runner(key="nc", builder=None):
    """Build (once) a cached jax.jit shard_map callable over the 8 cores."""
    rkey = "runner_" + key
    if rkey in _NC_CACHE:
        return _NC_CACHE[rkey]
    import jax
    import concourse.mybir as _mb
    from concourse import bass2jax as b2j
    from jax.sharding import Mesh, PartitionSpec
    from jax.experimental.shard_map import shard_map

    if key == "nc":
        nc = _get_module()
    else:
        if key not in _NC_CACHE:
            _NC_CACHE[key] = builder()
        nc = _NC_CACHE[key]
    b2j.install_neuronx_cc_hook()
    partition_name = (nc.partition_id_tensor.name
                      if nc.partition_id_tensor else None)
    in_names, out_names, out_avals, zero_outs = [], [], [], []
    for alloc in nc.m.functions[0].allocations:
        if not isinstance(alloc, _mb.MemoryLocationSet):
            continue
        name = alloc.memorylocations[0].name
        if alloc.kind == "ExternalInput":
            if name != partition_name:
                in_names.append(name)
        elif alloc.kind == "ExternalOutput":
            out_names.append(name)
            shape = tuple(alloc.tensor_shape)
            dtype = _mb.dt.np(alloc.dtype)
            out_avals.append(jax.core.ShapedArray(shape, dtype))
            zero_outs.append(np.zeros(shape, dtype))
    n_params = len(in_names)
    all_names = list(in_names) + list(out_names)
    if partition_name is not None:
        all_names.append(partition_name)

    def _body_fn(*args):
        operands = list(args)
        if partition_name is not None:
            operands.append(b2j.partition_id_tensor())
        outs = b2j._bass_exec_p.bind(
            *operands,
            out_avals=tuple(out_avals),
            in_names=tuple(all_names),
            out_names=tuple(out_names),
            lowering_input_output_aliases=(),
            sim_require_finite=True,
            sim_require_nnan=True,
            nc=nc,
        )
        return tuple(outs)

    devices = jax.devices()[:NCORES]
    mesh = Mesh(np.asarray(devices), ("core",))
    n_outs = len(out_names)
    in_specs = (PartitionSpec("core"),) * (n_params + n_outs)
    out_specs = (PartitionSpec("core"),) * n_outs
    sharded = jax.jit(
        shard_map(_body_fn, mesh=mesh, in_specs=in_specs,
                  out_specs=out_specs, check_rep=False),
        keep_unused=True)
    from jax.sharding import NamedSharding
    _shard = NamedSharding(mesh, PartitionSpec("core"))
    concat_zeros = [
        jax.device_put(
            np.zeros((NCORES * z.shape[0], *z.shape[1:]), z.dtype), _shard)
        for z in zero_outs
    ]
    runner = {
        "sharded": sharded, "in_names": in_names, "out_names": out_names,
        "out_avals": out_avals, "concat_zeros": concat_zeros,
    }
    _NC_CACHE[rkey] = runner
    return runner


_CONST_NAMES = {"trig", "consts"}


def _run_spmd_cached(in_maps):
    import jax
    r = _get_runner()
    ckey = "const_dev"
    if ckey not in _NC_CACHE:
        _NC_CACHE[ckey] = {}
    const_dev = _NC_CACHE[ckey]
    concat_in = []
    for nm in r["in_names"]:
        if nm in _CONST_NAMES:
            if nm not in const_dev:
                arr = np.concatenate(
                    [np.asarray(in_maps[c][nm]) for c in range(NCORES)],
                    axis=0)
                const_dev[nm] = jax.device_put(arr)
            concat_in.append(const_dev[nm])
        else:
            concat_in.append(np.concatenate(
                [np.asarray(in_maps[c][nm]) for c in range(NCORES)], axis=0))
    out_arrs = r["sharded"](*concat_in, *r["concat_zeros"])
    nm = r["out_names"]
    av = r["out_avals"]
    return [
        {nm[i]: np.asarray(out_arrs[i]).reshape(NCORES, *av[i].shape)[c]
         for i in range(len(nm))}
        for c in range(NCORES)
    ]


def kernel(x, wq, wk, wv, wo, _trace=False, _trace_kwargs=None):
    x = np.asarray(x, dtype=np.float32)
    wq = np.asarray(wq, dtype=np.float32)
    wk = np.asarray(wk, dtype=np.float32)
    wv = np.asarray(wv, dtype=np.float32)
    wo = np.asarray(wo, dtype=np.float32)

    in_maps = _prep_core_inputs(x, wq, wk, wv, wo)
    try:
        results = _run_spmd_cached(in_maps)
    except Exception:
        nc = _get_module()
        results = run_bass_kernel_spmd(
            nc, in_maps, core_ids=list(range(NCORES))).results
    out = np.empty((B, T, E), dtype=np.float32)
    for b_ in range(B):
        acc = np.zeros((E, T), dtype=np.float32)
        for g in range(4):
            acc += results[4 * b_ + g]["yt"]
        out[b_] = acc.T
    return out


if __name__ == "__main__":
    nc = _get_module()
    print("module built ok")



# revision 27
# speedup vs baseline: 1.0973x; 1.0973x over previous
"""Multi-head self-attention (16 heads, hd=64, RoPE, causal) on 8 trn2 cores.

Sharding: DP(batch=2) x TP(head-groups=4). Core c handles batch c//4, heads
[4*(c%4), 4*(c%4)+4). Each core computes a row-parallel partial output
yT_partial [1024, 2048] (bf16); host sums the 4 partials per batch in f32 and
transposes. No device-device communication.

Device kernel (v3):
  - bf16 x / wqkv / wo / q / k / v / trig / exp-weights / normalized
    attention; fp32 PSUM accumulation; bf16 output partials.
  - transposed layout throughout: xT [e,t], qT/kT [128, t] per head-pair
    (per-head rows laid out [e0:16|o0:16|e16:32|o16:32] so the RoPE partner
    swap is row^16 — one DVE stream_shuffle, no PE/DMA), scoresT [kt, q] per
    head, attnT via v_aug ones-column trick.
  - causal diag masking via gpsimd affine_select on the exp weights
    (no PE mask matmuls).
  - softmax denominators: DVE reciprocal on the psum ones-row + gpsimd
    partition_broadcast (no PE broadcast matmuls).
  - one exp activation per kt step covering both heads of a pair.
  - attention emitted as q-windows of 512 cols; a filler FIFO interleaves
    projection / output-projection matmuls between attention steps so the
    PE never idles (the cost model halves PE speed for 3us after any idle).
  - warmup matmuls on a zeroed tile bridge the initial DMA wait.
"""

import sys

for _p in ("/opt/trn_rl_repo",):
    if _p not in sys.path:
        sys.path.insert(0, _p)

from collections import deque
from contextlib import ExitStack

import numpy as np

import concourse.bass as bass
import concourse.mybir as mybir
import concourse.tile as tile
from concourse import bacc
from concourse.bass_utils import run_bass_kernel_spmd

F32 = mybir.dt.float32
BF16 = mybir.dt.bfloat16
AF = mybir.ActivationFunctionType
ALU = mybir.AluOpType

B, T, E = 2, 2048, 1024
NH, HD = 16, 64
NHL = 4          # heads per core
DL = NHL * HD    # 256 local head dims
NCORES = 8
ROPE_BASE = 10000.0

QW = 512         # attention q-window
NWIN = T // QW   # 4 windows
NBLK = 4         # projection t-blocks of 512
N_WARM = 27      # warmup matmuls (N=256) bridging the initial DMA wait

XOR16 = [i ^ 16 for i in range(32)]


# ----------------------------------------------------------------- device IR
def build_module(reps=1):
    nc = bacc.Bacc("TRN2", target_bir_lowering=False, debug=False,
                   num_devices=NCORES)

    xt = nc.dram_tensor("xt", [E, T], BF16, kind="ExternalInput").ap()
    wqkv = nc.dram_tensor("wqkv", [E, 3 * DL], BF16, kind="ExternalInput").ap()
    wot = nc.dram_tensor("wot", [DL, E], BF16, kind="ExternalInput").ap()
    trig = nc.dram_tensor("trig", [2, 128, T], BF16, kind="ExternalInput").ap()
    yt = nc.dram_tensor("yt", [E, T], BF16, kind="ExternalOutput").ap()

    with tile.TileContext(nc) as tc:
        for _ in range(reps):
            _body(tc, xt, wqkv, wot, trig, yt)
    nc.compile()
    return nc


def _body(tc, xt, wqkv, wot, trig, yt):
    nc = tc.nc

    with ExitStack() as ctx:
        po = ctx.enter_context(tc.tile_pool(name="po", bufs=1))
        xcp = ctx.enter_context(tc.tile_pool(name="xcp", bufs=8))
        rp = ctx.enter_context(tc.tile_pool(name="rp", bufs=5))
        ep = ctx.enter_context(tc.tile_pool(name="ep", bufs=4))
        dp = ctx.enter_context(tc.tile_pool(name="dp", bufs=4))
        yp = ctx.enter_context(tc.tile_pool(name="yp", bufs=2))
        pjp = ctx.enter_context(tc.tile_pool(name="pjp", bufs=2, space="PSUM"))
        ssp = ctx.enter_context(tc.tile_pool(name="ssp", bufs=2, space="PSUM"))
        sap = ctx.enter_context(tc.tile_pool(name="sap", bufs=2, space="PSUM"))

        # ---------------- persistent tiles --------------------------------
        # qk[0]=q pair0, qk[1]=q pair1, qk[2]=k pair0, qk[3]=k pair1
        qk = [po.tile([128, T], BF16, tag=f"qk{i}", name=f"qk{i}")
              for i in range(4)]
        # per (kt, local-head) slot of 128 lhsT columns: local head hh=2P+h
        # holds [v(64) | ones(64)] for h==0 and [ones(64) | v(64)] for h==1,
        # so the attnv matmul materializes the softmax denominator broadcast
        # across 64 psum rows for free (PE cost depends on moving dim only)
        v_sb = po.tile([128, 16 * 512], BF16, tag="v", name="v_sb")
        w_sb = po.tile([128, 8 * 768], BF16, tag="w", name="w_sb")
        wot_sb = [po.tile([128, E], BF16, tag=f"wot{p}", name=f"wot{p}")
                  for p in range(2)]
        trigc = po.tile([128, T], BF16, tag="tc", name="trigc")
        trigs = po.tile([128, T], BF16, tag="tsn", name="trigs")
        at = [po.tile([128, T], BF16, tag=f"at{p}", name=f"at{p}")
              for p in range(2)]
        warm = po.tile([128, 392], BF16, tag="warm", name="warm")

        # ---------------- init: memsets + DMAs -----------------------------
        nc.gpsimd.memset(warm[:], 0.0)
        # every (kt, head) slot is [ones(0:64) | v(64:128)]: the attnv matmul
        # then yields D on psum rows 0:64 and the numerator on rows 64:128
        v_ones = v_sb[:].rearrange("p (kt pp h j) -> p kt pp h j",
                                   kt=16, pp=2, h=2)
        nc.gpsimd.memset(v_ones[:, :, :, :, 0:64], 1.0)

        w_v = w_sb[:].rearrange("p (eo d) -> p eo d", eo=8)
        wqkv_v = wqkv.rearrange("(eo p) d -> p eo d", p=128)

        xc = {}

        def load_x(b, half):
            """one DMA for eo in [4*half, 4*half+4) of block b."""
            t_ = xcp.tile([128, 4 * 512], BF16, tag="xc", name="xc")
            tv = t_[:].rearrange("p (eo t) -> p eo t", eo=4)
            nc.sync.dma_start(
                out=tv,
                in_=xt[half * 512:(half + 1) * 512,
                       b * 512:(b + 1) * 512]
                .rearrange("(eo p) t -> p eo t", p=128))
            for i in range(4):
                xc[(b, 4 * half + i)] = tv[:, i, :]

        # issue order tuned so each transfer lands just before first use
        load_x(0, 0)
        nc.sync.dma_start(out=w_v[:, 0:4, 0:256], in_=wqkv_v[:, 0:4, 0:256])
        load_x(0, 1)
        nc.sync.dma_start(out=w_v[:, 4:8, 0:256], in_=wqkv_v[:, 4:8, 0:256])
        nc.sync.dma_start(out=w_v[:, :, 256:512], in_=wqkv_v[:, :, 256:512])
        nc.sync.dma_start(out=trigc[:, 0:512], in_=trig[0][:, 0:512])
        nc.sync.dma_start(out=trigs[:, 0:512], in_=trig[1][:, 0:512])
        nc.sync.dma_start(out=w_v[:, :, 512:768], in_=wqkv_v[:, :, 512:768])
        load_x(1, 0)
        load_x(1, 1)
        nc.sync.dma_start(out=trigc[:, 512:T], in_=trig[0][:, 512:T])
        nc.sync.dma_start(out=trigs[:, 512:T], in_=trig[1][:, 512:T])
        for p in range(2):
            nc.sync.dma_start(out=wot_sb[p][:],
                              in_=wot[p * 128:(p + 1) * 128, :])
        load_x(2, 0)
        load_x(2, 1)
        load_x(3, 0)
        load_x(3, 1)

        # activation-table load lands during the DMA wait (writes to a col
        # outside the warm matmul operand ranges so it doesn't gate them)
        nc.scalar.activation(warm[0:1, 384:385], warm[0:1, 0:1], AF.Exp)

        # warmup: keep PE busy (and ramping) until the first x chunks land
        warm_ps = pjp.tile([128, 256], F32, tag="pj", name="warm_ps")
        for i in range(N_WARM):
            nc.tensor.matmul(out=warm_ps[:], lhsT=warm[:, 0:128],
                             rhs=warm[:, 128:384],
                             start=(i == 0), stop=(i == N_WARM - 1))

        # ---------------- projection + rope emission helpers ---------------
        # nm: 0=q0, 1=q1, 2=k0, 3=k1 ; block b covers t cols [512b, 512b+512)
        def qk_mms(nm, b):
            """8 accumulating MMs + psum->bf16 copy; returns raw tile."""
            wcol = (nm % 2) * 128 if nm < 2 else 256 + (nm % 2) * 128
            ps = pjp.tile([128, 512], F32, tag="pj", name="pjqk")
            for eo in range(8):
                nc.tensor.matmul(
                    out=ps[:],
                    lhsT=w_sb[:, eo * 768 + wcol: eo * 768 + wcol + 128],
                    rhs=xc[(b, eo)][:],
                    start=(eo == 0), stop=(eo == 7))
            raw = rp.tile([128, 512], BF16, tag="raw", name="raw")
            if b == 0 and nm < 2:
                nc.scalar.copy(raw[:], ps[:])   # ACT is idle pre-attention
            else:
                nc.vector.tensor_copy(raw[:], ps[:])
            return raw

        def qk_rope(nm, b, raw):
            """row^16 partner swap (DVE shuffle) + cos/sin muls + add."""
            cs = slice(b * 512, b * 512 + 512)
            swp = rp.tile([128, 512], BF16, tag="swp", name="swp")
            nc.vector.stream_shuffle(swp[:], raw[:], XOR16)
            nc.gpsimd.tensor_mul(qk[nm][:, cs], raw[:], trigc[:, cs])
            tmp = rp.tile([128, 512], BF16, tag="tmp", name="tmp")
            nc.vector.tensor_mul(tmp[:], swp[:], trigs[:, cs])
            nc.vector.tensor_add(qk[nm][:, cs], qk[nm][:, cs], tmp[:])

        def v_grp(b, tt):
            """one 128-t-row V projection group; kt block = 4b+tt."""
            ps = pjp.tile([128, 256], F32, tag="pj", name="pjv")
            for eo in range(8):
                nc.tensor.matmul(
                    out=ps[:],
                    lhsT=xc[(b, eo)][:, tt * 128:tt * 128 + 128],
                    rhs=w_v[:, eo, 512:768],
                    start=(eo == 0), stop=(eo == 7))
            kt = 4 * b + tt
            slot4 = v_sb[:, kt * 512:(kt + 1) * 512] \
                .rearrange("p (pp h j) -> p pp h j", pp=2, h=2)
            ps4 = ps[:].rearrange("p (pp h x) -> p pp h x", pp=2, h=2)
            if b == 0 and tt % 2 == 0:
                nc.scalar.copy(slot4[:, :, :, 64:128], ps4[:])
            else:
                nc.vector.tensor_copy(slot4[:, :, :, 64:128], ps4[:])

        # ---------------- filler FIFO --------------------------------------
        fifo = deque()
        emitted = set()
        # rough PE-time of each item kind, for the debt-based pump
        COSTS = {"qkA": 1700.0, "qkB": 0.0, "v": 900.0, "yt": 480.0}
        debt = [0.0]

        def enqueue_block(b, b0_order=False):
            raws = {}
            if b0_order:
                # DMA arrival order at startup: A's first, then v/B
                # interleaved (psum-ring WARs hide behind alternation)
                order = [("A", 0), ("A", 2), ("A", 1), ("A", 3),
                         ("B", 0), ("B", 2), ("v", 0), ("B", 1),
                         ("v", 1), ("B", 3), ("v", 2), ("v", 3)]
            else:
                order = [("A", 0), ("A", 2), ("B", 0), ("A", 1), ("B", 2),
                         ("A", 3), ("B", 1), ("B", 3),
                         ("v", 0), ("v", 1), ("v", 2), ("v", 3)]
            for kind, x in order:
                if kind == "A":
                    fifo.append((("qkA", x, b),
                                 lambda nm=x, b=b: raws.__setitem__(
                                     nm, qk_mms(nm, b))))
                elif kind == "B":
                    fifo.append((("qkB", x, b),
                                 lambda nm=x, b=b: qk_rope(nm, b,
                                                           raws.pop(nm))))
                else:
                    fifo.append((("v", b, x),
                                 lambda b=b, tt=x: v_grp(b, tt)))

        def emit_next():
            tag, fn = fifo.popleft()
            fn()
            emitted.add(tag)

        reserve = [0]

        def pump_ns(ns):
            debt[0] += ns
            while len(fifo) > reserve[0] and debt[0] >= COSTS[fifo[0][0][0]]:
                k = fifo[0][0][0]
                emit_next()
                debt[0] -= COSTS[k]

        def force(tag):
            if tag in emitted:
                return
            while fifo:
                t, _ = fifo[0]
                emit_next()
                if t == tag:
                    debt[0] = 0.0
                    return
            raise AssertionError(f"force: {tag} never enqueued")

        # ---------------- normalization helpers -----------------------------
        IDENT32 = list(range(32))
        ZERO32 = [0] * 32

        def norm_h(P, W, h, c0, c1, ps_a, snapshot=False):
            """normalize cols [c0,c1) of pair P window W head-half h."""
            cw = c1 - c0
            sl = slice(c0, c1)
            gsl = slice(W * 512 + c0, W * 512 + c1)
            if snapshot:
                # stage the completed psum columns to SBUF with one short
                # copy so later attnv steps' writes only WAR against the
                # copy, not the whole norm chain
                stg = dp.tile([128, 512], F32, tag="stg", name="stg")
                if h == 0:
                    nc.scalar.copy(stg[:, 0:cw], ps_a[h][:, sl])
                else:
                    nc.vector.tensor_copy(stg[:, 0:cw], ps_a[h][:, sl])
                ps_a = {h: stg}
                sl = slice(0, cw)
            # D on psum rows 0:64, numerator on rows 64:128 for both heads.
            # reciprocal_approx_fast only works at partition base 0.
            rcb = dp.tile([128, 512], F32, tag="rcb", name="rcb")
            nc.vector.reciprocal_approx_fast(out=rcb[0:64, 0:cw],
                                             in_=ps_a[h][0:64, sl])
            nc.vector.stream_shuffle(rcb[64:128, 0:cw], rcb[0:64, 0:cw],
                                     IDENT32)
            if h == 1:
                nc.vector.tensor_mul(at[P][64:128, gsl], ps_a[1][64:128, sl],
                                     rcb[64:128, 0:cw])
            else:
                a0n = dp.tile([128, 512], BF16, tag="a0n", name="a0n")
                nc.vector.tensor_mul(a0n[64:128, 0:cw], ps_a[0][64:128, sl],
                                     rcb[64:128, 0:cw])
                nc.vector.stream_shuffle(at[P][0:64, gsl], a0n[64:128, 0:cw],
                                         IDENT32)

        # ---------------- output projection ---------------------------------
        ytv = yt.rearrange("(et p) t -> p et t", p=128)

        def enqueue_yt(W, c0=0, c1=512):
            qcols = slice(W * 512 + c0, W * 512 + c1)
            cw = c1 - c0
            y_sb = yp.tile([128, 8 * 512], BF16, tag="ysb", name="y_sb")
            tail_piece = cw < 512
            nst = 1 if tail_piece else 4   # store granularity (ets)

            def yt_grp(et):
                ps_y = pjp.tile([128, 512], F32, tag="pj", name="ps_y")
                for p in range(2):
                    nc.tensor.matmul(
                        out=ps_y[:, 0:cw],
                        lhsT=wot_sb[p][:, et * 128:(et + 1) * 128],
                        rhs=at[p][:, qcols],
                        start=(p == 0), stop=(p == 1))
                ydst = y_sb[:, et * 512:et * 512 + cw]
                if tail_piece and et % 2 == 0:
                    # exp stream is finishing: ACT takes half the tail copies
                    nc.scalar.copy(ydst, ps_y[:, 0:cw])
                else:
                    nc.vector.tensor_copy(ydst, ps_y[:, 0:cw])
                if tail_piece:
                    # batch stores: ets 0-3, 4-6, then et7 alone so the
                    # final DMA chain after the last copy is short
                    groups = {5: (0, 6), 7: (6, 2)}
                    if et in groups:
                        e0, ne = groups[et]
                        src = (y_sb[:, e0 * 512:(e0 + ne) * 512]
                               .rearrange("p (et t) -> p et t", et=ne)[:, :, 0:cw]
                               if ne > 1 else ydst)
                        nc.sync.dma_start(out=ytv[:, e0:e0 + ne, qcols],
                                          in_=src)
                elif et % nst == nst - 1:
                    eg = et // nst
                    nc.sync.dma_start(
                        out=ytv[:, eg * nst:(eg + 1) * nst, qcols],
                        in_=y_sb[:, eg * nst * 512:(eg + 1) * nst * 512]
                        .rearrange("p (et t) -> p et t", et=nst))

            for et in range(8):
                fifo.append((("yt", W, et), lambda et=et: yt_grp(et)))

        # ---------------- attention ----------------------------------------
        def att_call(P, W, tail=False):
            """attention for pair P, q cols [512W, 512W+512)."""
            nkt = 4 * W + 4
            qcols = slice(W * 512, W * 512 + 512)
            # rope of q[P] block W and k[P] blocks <= W must be emitted
            force(("qkB", P, W))
            for bb in range(W + 1):
                force(("qkB", 2 + P, bb))

            ps_a = [sap.tile([128, 512], F32, tag="a", name="ps_a")
                    for _ in range(2)]
            exps = [None] * nkt

            def scores_step(kt):
                qs = max(0, 128 * kt - 512 * W)
                diag = kt >= 4 * W
                ss = ssp.tile([128, 1024], F32, tag="s", name="ss")
                for h in range(2):
                    nc.tensor.matmul(
                        out=ss[:, h * 512 + qs: h * 512 + 512],
                        lhsT=qk[2 + P][h * 64:h * 64 + 64,
                                       kt * 128:kt * 128 + 128],
                        rhs=qk[P][h * 64:h * 64 + 64, W * 512 + qs:
                                  W * 512 + 512],
                        start=True, stop=True,
                        tile_position=(h * 64, 0))
                e = ep.tile([128, 1024], BF16, tag="e", name="exp_t")
                e3 = e[:].rearrange("p (h c) -> p h c", h=2)[:, :, qs:512]
                s3 = ss[:].rearrange("p (h c) -> p h c", h=2)[:, :, qs:512]
                nc.scalar.activation(e3, s3, AF.Exp)
                if diag:
                    ed = e[:].rearrange("p (h c) -> p h c",
                                        h=2)[:, :, qs:qs + 128]
                    nc.gpsimd.affine_select(
                        ed, ed, [[0, 2], [1, 128]], ALU.is_ge, 0.0,
                        base=0, channel_multiplier=-1)
                exps[kt] = (e, qs)

            def attnv_step(kt):
                e, qs = exps[kt]
                for h in range(2):
                    slot = kt * 512 + (2 * P + h) * 128
                    nc.tensor.matmul(
                        out=ps_a[h][0:128, qs:512],
                        lhsT=v_sb[:, slot:slot + 128],
                        rhs=e[:, h * 512 + qs: h * 512 + 512],
                        start=(kt == 0), stop=(kt == nkt - 1))
                exps[kt] = None

            for step in range(nkt + 1):
                if step < nkt:
                    # pre-force v blocks one block ahead of the kt cursor
                    vb = min(step // 4 + 1, W)
                    for bb in range(vb + 1):
                        for tt in range(4):
                            if (("v", bb, tt)) not in emitted:
                                force(("v", bb, tt))
                    scores_step(step)
                if step > 0:
                    attnv_step(step - 1)
                if tail and step == nkt - 2:
                    # attnv of kt=nkt-3 is done: window cols 0:256 complete;
                    # normalize them and make their yt groups available so
                    # the PE has real work through the final steps
                    norm_h(P, W, 1, 0, 256, ps_a, snapshot=True)
                    norm_h(P, W, 0, 0, 256, ps_a, snapshot=True)
                    enqueue_yt(W, 0, 256)
                    reserve[0] = 0   # norm latency window: release holdbacks
                # ACT-vs-PE imbalance this step, paid to the filler pump
                qs = max(0, 128 * min(step, nkt - 1) - 512 * W)
                cols = 512 - qs
                gap = (2 * cols * 0.833 + 500.0) - (4 * cols * 0.4167 + 107.0)
                pump_ns(max(200.0, gap))

            # ---------------- normalization -------------------------------
            # h1 first: its cross-partition shuffle is the longest pole
            if tail:
                norm_h(P, W, 1, 256, 512, ps_a)
                norm_h(P, W, 0, 256, 512, ps_a)
                enqueue_yt(W, 256, 512)
            else:
                pump_ns(600.0)
                norm_h(P, W, 1, 0, 512, ps_a)
                pump_ns(600.0)
                norm_h(P, W, 0, 0, 512, ps_a)
                pump_ns(600.0)

        # ---------------- master schedule -----------------------------------
        # block 0 emitted straight; blocks 1..3 via the FIFO
        enqueue_block(0, b0_order=True)
        while fifo:
            emit_next()
        for b in range(1, NBLK):
            enqueue_block(b)

        RESV = {(0, 2): 8, (1, 2): 8, (0, 3): 14, (1, 3): 5}
        for W in range(NWIN):
            for P in range(2):
                reserve[0] = RESV.get((P, W), 0)
                att_call(P, W, tail=(P == 1 and W == NWIN - 1))
            if W < NWIN - 1:
                enqueue_yt(W)
        while fifo:
            emit_next()


# ----------------------------------------------------------------- host side
def _prep_core_inputs(x, wq, wk, wv, wo):
    """Build the 8 per-core input dicts."""
    import ml_dtypes
    bf = ml_dtypes.bfloat16

    inv_freq = 1.0 / (ROPE_BASE ** (np.arange(0, HD, 2, dtype=np.float32) / HD))
    pos = np.arange(T, dtype=np.float32)
    freqs = pos[:, None] * inv_freq[None, :]          # [T, 32]
    cosT = np.cos(freqs).T.astype(np.float32)          # [32, T]
    sinT = np.sin(freqs).T.astype(np.float32)
    # per-head 64-row layout [e0:16 | o0:16 | e16:32 | o16:32] (partner = r^16)
    C64 = np.concatenate([cosT[0:16], cosT[0:16], cosT[16:32], cosT[16:32]])
    S64 = np.concatenate([-sinT[0:16], sinT[0:16], -sinT[16:32], sinT[16:32]])
    C = np.tile(C64, (2, 1))                           # [128, T]
    S = np.tile(S64, (2, 1))
    trig = np.stack([C, S]).astype(bf)                 # [2, 128, T]
    scale = np.float32(1.0 / np.sqrt(HD))              # folded into wq

    evens = np.arange(0, HD, 2)
    odds = np.arange(1, HD, 2)
    perm64 = np.concatenate([evens[0:16], odds[0:16], evens[16:32],
                             odds[16:32]])

    xts = [np.ascontiguousarray(x[b_].T).astype(bf) for b_ in range(B)]
    in_maps = []
    for core in range(NCORES):
        b_, hg = divmod(core, 4)
        heads = np.arange(4 * hg, 4 * hg + 4)
        qk_rows = np.concatenate([h * HD + perm64 for h in heads])
        v_rows = np.concatenate([h * HD + np.arange(HD) for h in heads])
        wq_t = wq.T[:, qk_rows] * scale                # [E, 256]
        wk_t = wk.T[:, qk_rows]
        wv_t = wv.T[:, v_rows]
        wqkv = np.concatenate([wq_t, wk_t, wv_t], axis=1).astype(bf)
        wot_ = np.ascontiguousarray(wo.T[v_rows, :]).astype(bf)
        in_maps.append({
            "xt": xts[b_], "wqkv": wqkv, "wot": wot_,
            "trig": trig,
        })
    return in_maps


_NC_CACHE = {}


def _get_module():
    if "nc" not in _NC_CACHE:
        _NC_CACHE["nc"] = build_module()
    return _NC_CACHE["nc"]


def _get_runner(key="nc", builder=None):
    """Build (once) a cached jax.jit shard_map callable over the 8 cores."""
    rkey = "runner_" + key
    if rkey in _NC_CACHE:
        return _NC_CACHE[rkey]
    import jax
    import concourse.mybir as _mb
    from concourse import bass2jax as b2j
    from jax.sharding import Mesh, PartitionSpec
    from jax.experimental.shard_map import shard_map

    if key == "nc":
        nc = _get_module()
    else:
        if key not in _NC_CACHE:
            _NC_CACHE[key] = builder()
        nc = _NC_CACHE[key]
    b2j.install_neuronx_cc_hook()
    partition_name = (nc.partition_id_tensor.name
                      if nc.partition_id_tensor else None)
    in_names, out_names, out_avals, zero_outs = [], [], [], []
    for alloc in nc.m.functions[0].allocations:
        if not isinstance(alloc, _mb.MemoryLocationSet):
            continue
        name = alloc.memorylocations[0].name
        if alloc.kind == "ExternalInput":
            if name != partition_name:
                in_names.append(name)
        elif alloc.kind == "ExternalOutput":
            out_names.append(name)
            shape = tuple(alloc.tensor_shape)
            dtype = _mb.dt.np(alloc.dtype)
            out_avals.append(jax.core.ShapedArray(shape, dtype))
            zero_outs.append(np.zeros(shape, dtype))
    n_params = len(in_names)
    all_names = list(in_names) + list(out_names)
    if partition_name is not None:
        all_names.append(partition_name)

    def _body_fn(*args):
        operands = list(args)
        if partition_name is not None:
            operands.append(b2j.partition_id_tensor())
        outs = b2j._bass_exec_p.bind(
            *operands,
            out_avals=tuple(out_avals),
            in_names=tuple(all_names),
            out_names=tuple(out_names),
            lowering_input_output_aliases=(),
            sim_require_finite=True,
            sim_require_nnan=True,
            nc=nc,
        )
        return tuple(outs)

    devices = jax.devices()[:NCORES]
    mesh = Mesh(np.asarray(devices), ("core",))
    n_outs = len(out_names)
    in_specs = (PartitionSpec("core"),) * (n_params + n_outs)
    out_specs = (PartitionSpec("core"),) * n_outs
    sharded = jax.jit(
        shard_map(_body_fn, mesh=mesh, in_specs=in_specs,
                  out_specs=out_specs, check_rep=False),
        keep_unused=True)
    from jax.sharding import NamedSharding
    _shard = NamedSharding(mesh, PartitionSpec("core"))
    concat_zeros = [
        jax.device_put(
            np.zeros((NCORES * z.shape[0], *z.shape[1:]), z.dtype), _shard)
        for z in zero_outs
    ]
    runner = {
        "sharded": sharded, "in_names": in_names, "out_names": out_names,
        "out_avals": out_avals, "concat_zeros": concat_zeros,
    }
    _NC_CACHE[rkey] = runner
    return runner


_CONST_NAMES = {"trig"}


def _run_spmd_cached(in_maps):
    import jax
    r = _get_runner()
    ckey = "const_dev"
    if ckey not in _NC_CACHE:
        _NC_CACHE[ckey] = {}
    const_dev = _NC_CACHE[ckey]
    concat_in = []
    for nm in r["in_names"]:
        if nm in _CONST_NAMES:
            if nm not in const_dev:
                arr = np.concatenate(
                    [np.asarray(in_maps[c][nm]) for c in range(NCORES)],
                    axis=0)
                const_dev[nm] = jax.device_put(arr)
            concat_in.append(const_dev[nm])
        else:
            concat_in.append(np.concatenate(
                [np.asarray(in_maps[c][nm]) for c in range(NCORES)], axis=0))
    out_arrs = r["sharded"](*concat_in, *r["concat_zeros"])
    nm = r["out_names"]
    av = r["out_avals"]
    return [
        {nm[i]: np.asarray(out_arrs[i]).reshape(NCORES, *av[i].shape)[c]
         for i in range(len(nm))}
        for c in range(NCORES)
    ]


def kernel(x, wq, wk, wv, wo, _trace=False, _trace_kwargs=None):
    x = np.asarray(x, dtype=np.float32)
    wq = np.asarray(wq, dtype=np.float32)
    wk = np.asarray(wk, dtype=np.float32)
    wv = np.asarray(wv, dtype=np.float32)
    wo = np.asarray(wo, dtype=np.float32)

    in_maps = _prep_core_inputs(x, wq, wk, wv, wo)
    try:
        results = _run_spmd_cached(in_maps)
    except Exception:
        nc = _get_module()
        results = run_bass_kernel_spmd(
            nc, in_maps, core_ids=list(range(NCORES))).results
    out = np.empty((B, T, E), dtype=np.float32)
    for b_ in range(B):
        acc = np.zeros((E, T), dtype=np.float32)
        for g in range(4):
            acc += results[4 * b_ + g]["yt"].astype(np.float32)
        out[b_] = acc.T
    return out


if __name__ == "__main__":
    nc = _get_module()
    print("module built ok")


# revision 33
# speedup vs baseline: 1.1016x; 1.0038x over previous
"""Multi-head self-attention (16 heads, hd=64, RoPE, causal) on 8 trn2 cores.

Sharding: DP(batch=2) x TP(head-groups=4). Core c handles batch c//4, heads
[4*(c%4), 4*(c%4)+4). Each core computes a row-parallel partial output
yT_partial [1024, 2048] (bf16); host sums the 4 partials per batch in f32 and
transposes. No device-device communication.

Device kernel (v3):
  - bf16 x / wqkv / wo / q / k / v / trig / exp-weights / normalized
    attention; fp32 PSUM accumulation; bf16 output partials.
  - transposed layout throughout: xT [e,t], qT/kT [128, t] per head-pair
    (per-head rows laid out [e0:16|o0:16|e16:32|o16:32] so the RoPE partner
    swap is row^16 — one DVE stream_shuffle, no PE/DMA), scoresT [kt, q] per
    head, attnT via v_aug ones-column trick.
  - causal diag masking via gpsimd affine_select on the exp weights
    (no PE mask matmuls).
  - softmax denominators: DVE reciprocal on the psum ones-row + gpsimd
    partition_broadcast (no PE broadcast matmuls).
  - one exp activation per kt step covering both heads of a pair.
  - attention emitted as q-windows of 512 cols; a filler FIFO interleaves
    projection / output-projection matmuls between attention steps so the
    PE never idles (the cost model halves PE speed for 3us after any idle).
  - warmup matmuls on a zeroed tile bridge the initial DMA wait.
"""

import sys

for _p in ("/opt/trn_rl_repo",):
    if _p not in sys.path:
        sys.path.insert(0, _p)

from collections import deque
from contextlib import ExitStack

import numpy as np

import concourse.bass as bass
import concourse.mybir as mybir
import concourse.tile as tile
from concourse import bacc
from concourse.bass_utils import run_bass_kernel_spmd

F32 = mybir.dt.float32
BF16 = mybir.dt.bfloat16
AF = mybir.ActivationFunctionType
ALU = mybir.AluOpType

B, T, E = 2, 2048, 1024
NH, HD = 16, 64
NHL = 4          # heads per core
DL = NHL * HD    # 256 local head dims
NCORES = 8
ROPE_BASE = 10000.0

QW = 512         # attention q-window
NWIN = T // QW   # 4 windows
NBLK = 4         # projection t-blocks of 512
N_WARM = 22      # warmup matmuls (N=256) bridging the initial DMA wait

XOR16 = [i ^ 16 for i in range(32)]


# ----------------------------------------------------------------- device IR
def build_module(reps=1):
    nc = bacc.Bacc("TRN2", target_bir_lowering=False, debug=False,
                   num_devices=NCORES)

    xt = nc.dram_tensor("xt", [E, T], BF16, kind="ExternalInput").ap()
    wqkv = nc.dram_tensor("wqkv", [E, 3 * DL], BF16, kind="ExternalInput").ap()
    wot = nc.dram_tensor("wot", [DL, E], BF16, kind="ExternalInput").ap()
    trig = nc.dram_tensor("trig", [2, 128, T], BF16, kind="ExternalInput").ap()
    yt = nc.dram_tensor("yt", [E, T], BF16, kind="ExternalOutput").ap()

    with tile.TileContext(nc) as tc:
        for _ in range(reps):
            _body(tc, xt, wqkv, wot, trig, yt)
    nc.compile()
    return nc


def _body(tc, xt, wqkv, wot, trig, yt):
    nc = tc.nc

    with ExitStack() as ctx:
        po = ctx.enter_context(tc.tile_pool(name="po", bufs=1))
        xcp = ctx.enter_context(tc.tile_pool(name="xcp", bufs=8))
        rp = ctx.enter_context(tc.tile_pool(name="rp", bufs=5))
        ep = ctx.enter_context(tc.tile_pool(name="ep", bufs=4))
        dp = ctx.enter_context(tc.tile_pool(name="dp", bufs=6))
        yp = ctx.enter_context(tc.tile_pool(name="yp", bufs=2))
        pjp = ctx.enter_context(tc.tile_pool(name="pjp", bufs=2, space="PSUM"))
        ssp = ctx.enter_context(tc.tile_pool(name="ssp", bufs=2, space="PSUM"))
        sap = ctx.enter_context(tc.tile_pool(name="sap", bufs=2, space="PSUM"))

        # ---------------- persistent tiles --------------------------------
        # qk[0]=q pair0, qk[1]=q pair1, qk[2]=k pair0, qk[3]=k pair1
        qk = [po.tile([128, T], BF16, tag=f"qk{i}", name=f"qk{i}")
              for i in range(4)]
        # per (kt, local-head) slot of 128 lhsT columns: local head hh=2P+h
        # holds [v(64) | ones(64)] for h==0 and [ones(64) | v(64)] for h==1,
        # so the attnv matmul materializes the softmax denominator broadcast
        # across 64 psum rows for free (PE cost depends on moving dim only)
        v_sb = po.tile([128, 16 * 512], BF16, tag="v", name="v_sb")
        w_sb = po.tile([128, 8 * 768], BF16, tag="w", name="w_sb")
        wot_sb = [po.tile([128, E], BF16, tag=f"wot{p}", name=f"wot{p}")
                  for p in range(2)]
        trigc = po.tile([128, T], BF16, tag="tc", name="trigc")
        trigs = po.tile([128, T], BF16, tag="tsn", name="trigs")
        at = [po.tile([128, T], BF16, tag=f"at{p}", name=f"at{p}")
              for p in range(2)]
        warm = po.tile([128, 392], BF16, tag="warm", name="warm")

        # ---------------- init: memsets + DMAs -----------------------------
        nc.gpsimd.memset(warm[:], 0.0)
        # every (kt, head) slot is [ones(0:64) | v(64:128)]: the attnv matmul
        # then yields D on psum rows 0:64 and the numerator on rows 64:128
        v_ones = v_sb[:].rearrange("p (kt pp h j) -> p kt pp h j",
                                   kt=16, pp=2, h=2)
        nc.gpsimd.memset(v_ones[:, :, :, :, 0:64], 1.0)

        w_v = w_sb[:].rearrange("p (eo d) -> p eo d", eo=8)
        wqkv_v = wqkv.rearrange("(eo p) d -> p eo d", p=128)

        xc = {}

        def load_x(b, half):
            """one DMA for eo in [4*half, 4*half+4) of block b."""
            t_ = xcp.tile([128, 4 * 512], BF16, tag="xc", name="xc")
            tv = t_[:].rearrange("p (eo t) -> p eo t", eo=4)
            nc.sync.dma_start(
                out=tv,
                in_=xt[half * 512:(half + 1) * 512,
                       b * 512:(b + 1) * 512]
                .rearrange("(eo p) t -> p eo t", p=128))
            for i in range(4):
                xc[(b, 4 * half + i)] = tv[:, i, :]

        # issue order tuned so each transfer lands just before first use
        load_x(0, 0)
        nc.sync.dma_start(out=w_v[:, 0:4, 0:256], in_=wqkv_v[:, 0:4, 0:256])
        load_x(0, 1)
        nc.sync.dma_start(out=w_v[:, 4:8, 0:256], in_=wqkv_v[:, 4:8, 0:256])
        nc.sync.dma_start(out=w_v[:, :, 256:512], in_=wqkv_v[:, :, 256:512])
        nc.sync.dma_start(out=trigc[:, 0:512], in_=trig[0][:, 0:512])
        nc.sync.dma_start(out=trigs[:, 0:512], in_=trig[1][:, 0:512])
        nc.sync.dma_start(out=w_v[:, :, 512:768], in_=wqkv_v[:, :, 512:768])
        load_x(1, 0)
        load_x(1, 1)
        nc.sync.dma_start(out=trigc[:, 512:T], in_=trig[0][:, 512:T])
        nc.sync.dma_start(out=trigs[:, 512:T], in_=trig[1][:, 512:T])
        for p in range(2):
            nc.sync.dma_start(out=wot_sb[p][:],
                              in_=wot[p * 128:(p + 1) * 128, :])
        load_x(2, 0)
        load_x(2, 1)
        load_x(3, 0)
        load_x(3, 1)

        # activation-table load lands during the DMA wait (writes to a col
        # outside the warm matmul operand ranges so it doesn't gate them)
        nc.scalar.activation(warm[0:1, 384:385], warm[0:1, 0:1], AF.Exp)

        # warmup: keep PE busy (and ramping) until the first x chunks land
        warm_ps = pjp.tile([128, 256], F32, tag="pj", name="warm_ps")
        for i in range(N_WARM):
            nc.tensor.matmul(out=warm_ps[:], lhsT=warm[:, 0:128],
                             rhs=warm[:, 128:384],
                             start=(i == 0), stop=(i == N_WARM - 1))

        # ---------------- projection + rope emission helpers ---------------
        # nm: 0=q0, 1=q1, 2=k0, 3=k1 ; block b covers t cols [512b, 512b+512)
        def qk_mms(nm, b):
            """8 accumulating MMs + psum->bf16 copy; returns raw tile."""
            wcol = (nm % 2) * 128 if nm < 2 else 256 + (nm % 2) * 128
            ps = pjp.tile([128, 512], F32, tag="pj", name="pjqk")
            for eo in range(8):
                nc.tensor.matmul(
                    out=ps[:],
                    lhsT=w_sb[:, eo * 768 + wcol: eo * 768 + wcol + 128],
                    rhs=xc[(b, eo)][:],
                    start=(eo == 0), stop=(eo == 7))
            raw = rp.tile([128, 512], BF16, tag="raw", name="raw")
            if b == 0 and nm < 2:
                nc.scalar.copy(raw[:], ps[:])   # ACT is idle pre-attention
            else:
                nc.vector.tensor_copy(raw[:], ps[:])
            return raw

        def qk_rope(nm, b, raw):
            """row^16 partner swap (DVE shuffle) + cos/sin muls + add."""
            cs = slice(b * 512, b * 512 + 512)
            swp = rp.tile([128, 512], BF16, tag="swp", name="swp")
            nc.vector.stream_shuffle(swp[:], raw[:], XOR16)
            nc.gpsimd.tensor_mul(qk[nm][:, cs], raw[:], trigc[:, cs])
            tmp = rp.tile([128, 512], BF16, tag="tmp", name="tmp")
            nc.vector.tensor_mul(tmp[:], swp[:], trigs[:, cs])
            nc.vector.tensor_add(qk[nm][:, cs], qk[nm][:, cs], tmp[:])

        def v_grp(b, tt):
            """one 128-t-row V projection group; kt block = 4b+tt."""
            ps = pjp.tile([128, 256], F32, tag="pj", name="pjv")
            for eo in range(8):
                nc.tensor.matmul(
                    out=ps[:],
                    lhsT=xc[(b, eo)][:, tt * 128:tt * 128 + 128],
                    rhs=w_v[:, eo, 512:768],
                    start=(eo == 0), stop=(eo == 7))
            kt = 4 * b + tt
            slot4 = v_sb[:, kt * 512:(kt + 1) * 512] \
                .rearrange("p (pp h j) -> p pp h j", pp=2, h=2)
            ps4 = ps[:].rearrange("p (pp h x) -> p pp h x", pp=2, h=2)
            if b == 0 and tt % 2 == 0:
                nc.scalar.copy(slot4[:, :, :, 64:128], ps4[:])
            else:
                nc.vector.tensor_copy(slot4[:, :, :, 64:128], ps4[:])

        # ---------------- filler FIFO --------------------------------------
        fifo = deque()
        emitted = set()
        # rough PE-time of each item kind, for the debt-based pump
        COSTS = {"qkA": 1700.0, "qkB": 0.0, "v": 900.0, "yt": 480.0}
        debt = [0.0]

        def enqueue_block(b, b0_order=False):
            raws = {}
            if b0_order:
                # DMA arrival order at startup: A's first, then v/B
                # interleaved (psum-ring WARs hide behind alternation)
                order = [("A", 0), ("A", 2), ("A", 1), ("A", 3),
                         ("B", 0), ("B", 2), ("v", 0), ("B", 1),
                         ("v", 1), ("B", 3), ("v", 2), ("v", 3)]
            else:
                order = [("A", 0), ("A", 2), ("B", 0), ("A", 1), ("B", 2),
                         ("A", 3), ("B", 1), ("B", 3),
                         ("v", 0), ("v", 1), ("v", 2), ("v", 3)]
            for kind, x in order:
                if kind == "A":
                    fifo.append((("qkA", x, b),
                                 lambda nm=x, b=b: raws.__setitem__(
                                     nm, qk_mms(nm, b))))
                elif kind == "B":
                    fifo.append((("qkB", x, b),
                                 lambda nm=x, b=b: qk_rope(nm, b,
                                                           raws.pop(nm))))
                else:
                    fifo.append((("v", b, x),
                                 lambda b=b, tt=x: v_grp(b, tt)))

        def emit_next():
            tag, fn = fifo.popleft()
            fn()
            emitted.add(tag)

        reserve = [0]

        def pump_ns(ns):
            debt[0] += ns
            while len(fifo) > reserve[0] and debt[0] >= COSTS[fifo[0][0][0]]:
                k = fifo[0][0][0]
                emit_next()
                debt[0] -= COSTS[k]

        def force(tag):
            if tag in emitted:
                return
            while fifo:
                t, _ = fifo[0]
                emit_next()
                if t == tag:
                    debt[0] = 0.0
                    return
            raise AssertionError(f"force: {tag} never enqueued")

        # ---------------- normalization helpers -----------------------------
        IDENT32 = list(range(32))
        ZERO32 = [0] * 32

        def norm_h(P, W, h, c0, c1, ps_a, snapshot=False):
            """normalize cols [c0,c1) of pair P window W head-half h."""
            cw = c1 - c0
            sl = slice(c0, c1)
            gsl = slice(W * 512 + c0, W * 512 + c1)
            if snapshot:
                # stage the completed psum columns to SBUF with one short
                # copy so later attnv steps' writes only WAR against the
                # copy, not the whole norm chain
                stg = dp.tile([128, 512], F32, tag="stg", name="stg")
                if h == 0:
                    nc.scalar.copy(stg[:, 0:cw], ps_a[h][:, sl])
                else:
                    nc.vector.tensor_copy(stg[:, 0:cw], ps_a[h][:, sl])
                ps_a = {h: stg}
                sl = slice(0, cw)
            # D on psum rows 0:64, numerator on rows 64:128 for both heads.
            # reciprocal_approx_fast only works at partition base 0.
            rcb = dp.tile([128, 512], F32, tag="rcb", name="rcb")
            nc.vector.reciprocal_approx_fast(out=rcb[0:64, 0:cw],
                                             in_=ps_a[h][0:64, sl])
            nc.vector.stream_shuffle(rcb[64:128, 0:cw], rcb[0:64, 0:cw],
                                     IDENT32)
            if h == 1:
                nc.vector.tensor_mul(at[P][64:128, gsl], ps_a[1][64:128, sl],
                                     rcb[64:128, 0:cw])
            else:
                a0n = dp.tile([128, 512], BF16, tag="a0n", name="a0n")
                nc.vector.tensor_mul(a0n[64:128, 0:cw], ps_a[0][64:128, sl],
                                     rcb[64:128, 0:cw])
                nc.vector.stream_shuffle(at[P][0:64, gsl], a0n[64:128, 0:cw],
                                         IDENT32)

        # ---------------- output projection ---------------------------------
        ytv = yt.rearrange("(et p) t -> p et t", p=128)

        def enqueue_yt(W, c0=0, c1=512):
            qcols = slice(W * 512 + c0, W * 512 + c1)
            cw = c1 - c0
            y_sb = yp.tile([128, 8 * 512], BF16, tag="ysb", name="y_sb")
            tail_piece = cw < 512
            nst = 1 if tail_piece else 4   # store granularity (ets)

            def yt_grp(et):
                ps_y = pjp.tile([128, 512], F32, tag="pj", name="ps_y")
                for p in range(2):
                    nc.tensor.matmul(
                        out=ps_y[:, 0:cw],
                        lhsT=wot_sb[p][:, et * 128:(et + 1) * 128],
                        rhs=at[p][:, qcols],
                        start=(p == 0), stop=(p == 1))
                ydst = y_sb[:, et * 512:et * 512 + cw]
                if tail_piece and et % 2 == 0:
                    # exp stream is finishing: ACT takes half the tail copies
                    nc.scalar.copy(ydst, ps_y[:, 0:cw])
                else:
                    nc.vector.tensor_copy(ydst, ps_y[:, 0:cw])
                if tail_piece:
                    # batch stores: ets 0-3, 4-6, then et7 alone so the
                    # final DMA chain after the last copy is short
                    groups = {5: (0, 6), 7: (6, 2)}
                    if et in groups:
                        e0, ne = groups[et]
                        src = (y_sb[:, e0 * 512:(e0 + ne) * 512]
                               .rearrange("p (et t) -> p et t", et=ne)[:, :, 0:cw]
                               if ne > 1 else ydst)
                        nc.sync.dma_start(out=ytv[:, e0:e0 + ne, qcols],
                                          in_=src)
                elif et % nst == nst - 1:
                    eg = et // nst
                    nc.sync.dma_start(
                        out=ytv[:, eg * nst:(eg + 1) * nst, qcols],
                        in_=y_sb[:, eg * nst * 512:(eg + 1) * nst * 512]
                        .rearrange("p (et t) -> p et t", et=nst))

            for et in range(8):
                fifo.append((("yt", W, et), lambda et=et: yt_grp(et)))

        # ---------------- attention ----------------------------------------
        def att_call(P, W, tail=False):
            """attention for pair P, q cols [512W, 512W+512)."""
            nkt = 4 * W + 4
            qcols = slice(W * 512, W * 512 + 512)
            # rope of q[P] block W and k[P] blocks <= W must be emitted
            force(("qkB", P, W))
            for bb in range(W + 1):
                force(("qkB", 2 + P, bb))

            ps_a = [sap.tile([128, 512], F32, tag="a", name="ps_a")
                    for _ in range(2)]
            exps = [None] * nkt

            def scores_step(kt):
                qs = max(0, 128 * kt - 512 * W)
                diag = kt >= 4 * W
                ss = ssp.tile([128, 1024], F32, tag="s", name="ss")
                for h in range(2):
                    nc.tensor.matmul(
                        out=ss[:, h * 512 + qs: h * 512 + 512],
                        lhsT=qk[2 + P][h * 64:h * 64 + 64,
                                       kt * 128:kt * 128 + 128],
                        rhs=qk[P][h * 64:h * 64 + 64, W * 512 + qs:
                                  W * 512 + 512],
                        start=True, stop=True,
                        tile_position=(h * 64, 0))
                e = ep.tile([128, 1024], BF16, tag="e", name="exp_t")
                e3 = e[:].rearrange("p (h c) -> p h c", h=2)[:, :, qs:512]
                s3 = ss[:].rearrange("p (h c) -> p h c", h=2)[:, :, qs:512]
                nc.scalar.activation(e3, s3, AF.Exp)
                if diag:
                    ed = e[:].rearrange("p (h c) -> p h c",
                                        h=2)[:, :, qs:qs + 128]
                    nc.gpsimd.affine_select(
                        ed, ed, [[0, 2], [1, 128]], ALU.is_ge, 0.0,
                        base=0, channel_multiplier=-1)
                exps[kt] = (e, qs)

            def attnv_step(kt):
                e, qs = exps[kt]
                for h in range(2):
                    slot = kt * 512 + (2 * P + h) * 128
                    nc.tensor.matmul(
                        out=ps_a[h][0:128, qs:512],
                        lhsT=v_sb[:, slot:slot + 128],
                        rhs=e[:, h * 512 + qs: h * 512 + 512],
                        start=(kt == 0), stop=(kt == nkt - 1))
                exps[kt] = None

            for step in range(nkt + 1):
                if step < nkt:
                    # pre-force v blocks one block ahead of the kt cursor
                    vb = min(step // 4 + 1, W)
                    for bb in range(vb + 1):
                        for tt in range(4):
                            if (("v", bb, tt)) not in emitted:
                                force(("v", bb, tt))
                    scores_step(step)
                if step > 0:
                    attnv_step(step - 1)
                if tail and step == nkt - 2:
                    # attnv of kt=nkt-3 is done: window cols 0:256 complete;
                    # normalize them and make their yt groups available so
                    # the PE has real work through the final steps
                    norm_h(P, W, 1, 0, 256, ps_a, snapshot=True)
                    norm_h(P, W, 0, 0, 256, ps_a, snapshot=True)
                    enqueue_yt(W, 0, 256)
                    reserve[0] = 0   # norm latency window: release holdbacks
                # ACT-vs-PE imbalance this step, paid to the filler pump
                qs = max(0, 128 * min(step, nkt - 1) - 512 * W)
                cols = 512 - qs
                gap = (2 * cols * 0.833 + 500.0) - (4 * cols * 0.4167 + 107.0)
                pump_ns(max(200.0, gap))

            # ---------------- normalization -------------------------------
            # h1 first: its cross-partition shuffle is the longest pole
            if tail:
                norm_h(P, W, 1, 256, 512, ps_a)
                norm_h(P, W, 0, 256, 512, ps_a)
                enqueue_yt(W, 256, 512)
            else:
                pump_ns(600.0)
                norm_h(P, W, 1, 0, 512, ps_a)
                pump_ns(600.0)
                norm_h(P, W, 0, 0, 512, ps_a)
                pump_ns(600.0)

        # ---------------- master schedule -----------------------------------
        # block 0 emitted straight; blocks 1..3 via the FIFO
        enqueue_block(0, b0_order=True)
        while fifo:
            emit_next()
        for b in range(1, NBLK):
            enqueue_block(b)

        RESV = {(0, 2): 8, (1, 2): 8, (0, 3): 14, (1, 3): 8}
        for W in range(NWIN):
            for P in range(2):
                reserve[0] = RESV.get((P, W), 0)
                att_call(P, W, tail=(P == 1 and W == NWIN - 1))
            if W < NWIN - 1:
                enqueue_yt(W)
        while fifo:
            emit_next()


# ----------------------------------------------------------------- host side
def _prep_core_inputs(x, wq, wk, wv, wo):
    """Build the 8 per-core input dicts."""
    import ml_dtypes
    bf = ml_dtypes.bfloat16

    inv_freq = 1.0 / (ROPE_BASE ** (np.arange(0, HD, 2, dtype=np.float32) / HD))
    pos = np.arange(T, dtype=np.float32)
    freqs = pos[:, None] * inv_freq[None, :]          # [T, 32]
    cosT = np.cos(freqs).T.astype(np.float32)          # [32, T]
    sinT = np.sin(freqs).T.astype(np.float32)
    # per-head 64-row layout [e0:16 | o0:16 | e16:32 | o16:32] (partner = r^16)
    C64 = np.concatenate([cosT[0:16], cosT[0:16], cosT[16:32], cosT[16:32]])
    S64 = np.concatenate([-sinT[0:16], sinT[0:16], -sinT[16:32], sinT[16:32]])
    C = np.tile(C64, (2, 1))                           # [128, T]
    S = np.tile(S64, (2, 1))
    trig = np.stack([C, S]).astype(bf)                 # [2, 128, T]
    scale = np.float32(1.0 / np.sqrt(HD))              # folded into wq

    evens = np.arange(0, HD, 2)
    odds = np.arange(1, HD, 2)
    perm64 = np.concatenate([evens[0:16], odds[0:16], evens[16:32],
                             odds[16:32]])

    xts = [np.ascontiguousarray(x[b_].T).astype(bf) for b_ in range(B)]
    in_maps = []
    for core in range(NCORES):
        b_, hg = divmod(core, 4)
        heads = np.arange(4 * hg, 4 * hg + 4)
        qk_rows = np.concatenate([h * HD + perm64 for h in heads])
        v_rows = np.concatenate([h * HD + np.arange(HD) for h in heads])
        wq_t = wq.T[:, qk_rows] * scale                # [E, 256]
        wk_t = wk.T[:, qk_rows]
        wv_t = wv.T[:, v_rows]
        wqkv = np.concatenate([wq_t, wk_t, wv_t], axis=1).astype(bf)
        wot_ = np.ascontiguousarray(wo.T[v_rows, :]).astype(bf)
        in_maps.append({
            "xt": xts[b_], "wqkv": wqkv, "wot": wot_,
            "trig": trig,
        })
    return in_maps


_NC_CACHE = {}


def _get_module():
    if "nc" not in _NC_CACHE:
        _NC_CACHE["nc"] = build_module()
    return _NC_CACHE["nc"]


def _get_runner(key="nc", builder=None):
    """Build (once) a cached jax.jit shard_map callable over the 8 cores."""
    rkey = "runner_" + key
    if rkey in _NC_CACHE:
        return _NC_CACHE[rkey]
    import jax
    import concourse.mybir as _mb
    from concourse import bass2jax as b2j
    from jax.sharding import Mesh, PartitionSpec
    from jax.experimental.shard_map import shard_map

    if key == "nc":
        nc = _get_module()
    else:
        if key not in _NC_CACHE:
            _NC_CACHE[key] = builder()
        nc = _NC_CACHE[key]
    b2j.install_neuronx_cc_hook()
    partition_name = (nc.partition_id_tensor.name
                      if nc.partition_id_tensor else None)
    in_names, out_names, out_avals, zero_outs = [], [], [], []
    for alloc in nc.m.functions[0].allocations:
        if not isinstance(alloc, _mb.MemoryLocationSet):
            continue
        name = alloc.memorylocations[0].name
        if alloc.kind == "ExternalInput":
            if name != partition_name:
                in_names.append(name)
        elif alloc.kind == "ExternalOutput":
            out_names.append(name)
            shape = tuple(alloc.tensor_shape)
            dtype = _mb.dt.np(alloc.dtype)
            out_avals.append(jax.core.ShapedArray(shape, dtype))
            zero_outs.append(np.zeros(shape, dtype))
    n_params = len(in_names)
    all_names = list(in_names) + list(out_names)
    if partition_name is not None:
        all_names.append(partition_name)

    def _body_fn(*args):
        operands = list(args)
        if partition_name is not None:
            operands.append(b2j.partition_id_tensor())
        outs = b2j._bass_exec_p.bind(
            *operands,
            out_avals=tuple(out_avals),
            in_names=tuple(all_names),
            out_names=tuple(out_names),
            lowering_input_output_aliases=(),
            sim_require_finite=True,
            sim_require_nnan=True,
            nc=nc,
        )
        return tuple(outs)

    devices = jax.devices()[:NCORES]
    mesh = Mesh(np.asarray(devices), ("core",))
    n_outs = len(out_names)
    in_specs = (PartitionSpec("core"),) * (n_params + n_outs)
    out_specs = (PartitionSpec("core"),) * n_outs
    sharded = jax.jit(
        shard_map(_body_fn, mesh=mesh, in_specs=in_specs,
                  out_specs=out_specs, check_rep=False),
        keep_unused=True)
    from jax.sharding import NamedSharding
    _shard = NamedSharding(mesh, PartitionSpec("core"))
    concat_zeros = [
        jax.device_put(
            np.zeros((NCORES * z.shape[0], *z.shape[1:]), z.dtype), _shard)
        for z in zero_outs
    ]
    runner = {
        "sharded": sharded, "in_names": in_names, "out_names": out_names,
        "out_avals": out_avals, "concat_zeros": concat_zeros,
    }
    _NC_CACHE[rkey] = runner
    return runner


_CONST_NAMES = {"trig"}


def _run_spmd_cached(in_maps):
    import jax
    r = _get_runner()
    ckey = "const_dev"
    if ckey not in _NC_CACHE:
        _NC_CACHE[ckey] = {}
    const_dev = _NC_CACHE[ckey]
    concat_in = []
    for nm in r["in_names"]:
        if nm in _CONST_NAMES:
            if nm not in const_dev:
                arr = np.concatenate(
                    [np.asarray(in_maps[c][nm]) for c in range(NCORES)],
                    axis=0)
                const_dev[nm] = jax.device_put(arr)
            concat_in.append(const_dev[nm])
        else:
            concat_in.append(np.concatenate(
                [np.asarray(in_maps[c][nm]) for c in range(NCORES)], axis=0))
    out_arrs = r["sharded"](*concat_in, *r["concat_zeros"])
    nm = r["out_names"]
    av = r["out_avals"]
    return [
        {nm[i]: np.asarray(out_arrs[i]).reshape(NCORES, *av[i].shape)[c]
         for i in range(len(nm))}
        for c in range(NCORES)
    ]


def kernel(x, wq, wk, wv, wo, _trace=False, _trace_kwargs=None):
    x = np.asarray(x, dtype=np.float32)
    wq = np.asarray(wq, dtype=np.float32)
    wk = np.asarray(wk, dtype=np.float32)
    wv = np.asarray(wv, dtype=np.float32)
    wo = np.asarray(wo, dtype=np.float32)

    in_maps = _prep_core_inputs(x, wq, wk, wv, wo)
    try:
        results = _run_spmd_cached(in_maps)
    except Exception:
        nc = _get_module()
        results = run_bass_kernel_spmd(
            nc, in_maps, core_ids=list(range(NCORES))).results
    out = np.empty((B, T, E), dtype=np.float32)
    for b_ in range(B):
        acc = np.zeros((E, T), dtype=np.float32)
        for g in range(4):
            acc += results[4 * b_ + g]["yt"].astype(np.float32)
        out[b_] = acc.T
    return out


if __name__ == "__main__":
    nc = _get_module()
    print("module built ok")


# revision 43
# speedup vs baseline: 1.1057x; 1.0038x over previous
"""Multi-head self-attention (16 heads, hd=64, RoPE, causal) on 8 trn2 cores.

Sharding: DP(batch=2) x TP(head-groups=4). Core c handles batch c//4, heads
[4*(c%4), 4*(c%4)+4). Each core computes a row-parallel partial output
yT_partial [1024, 2048] (bf16); host sums the 4 partials per batch in f32 and
transposes. No device-device communication.

Device kernel (v3):
  - bf16 x / wqkv / wo / q / k / v / trig / exp-weights / normalized
    attention; fp32 PSUM accumulation; bf16 output partials.
  - transposed layout throughout: xT [e,t], qT/kT [128, t] per head-pair
    (per-head rows laid out [e0:16|o0:16|e16:32|o16:32] so the RoPE partner
    swap is row^16 — one DVE stream_shuffle, no PE/DMA), scoresT [kt, q] per
    head, attnT via v_aug ones-column trick.
  - causal diag masking via gpsimd affine_select on the exp weights
    (no PE mask matmuls).
  - softmax denominators via 64 replicated ones-columns in the attnv lhsT
    (free: PE cost is moving-dim only), giving D pre-broadcast on psum rows
    0:64; DVE reciprocal (base 0 only!) + stream_shuffle cross-partition
    moves replace all PE broadcast matmuls and a1n staging DMAs.
  - last att_call normalizes 256-col slices as their diag steps complete
    (via an SBUF snapshot to keep the psum WAR short), so output-projection
    work overlaps the final attention steps and the tail stays short.
  - one exp activation per kt step covering both heads of a pair.
  - attention emitted as q-windows of 512 cols; a filler FIFO interleaves
    projection / output-projection matmuls between attention steps so the
    PE never idles (the cost model halves PE speed for 3us after any idle).
  - warmup matmuls on a zeroed tile bridge the initial DMA wait.
"""

import sys

for _p in ("/opt/trn_rl_repo",):
    if _p not in sys.path:
        sys.path.insert(0, _p)

from collections import deque
from contextlib import ExitStack

import numpy as np

import concourse.bass as bass
import concourse.mybir as mybir
import concourse.tile as tile
from concourse import bacc
from concourse.bass_utils import run_bass_kernel_spmd

F32 = mybir.dt.float32
BF16 = mybir.dt.bfloat16
AF = mybir.ActivationFunctionType
ALU = mybir.AluOpType

B, T, E = 2, 2048, 1024
NH, HD = 16, 64
NHL = 4          # heads per core
DL = NHL * HD    # 256 local head dims
NCORES = 8
ROPE_BASE = 10000.0

QW = 512         # attention q-window
NWIN = T // QW   # 4 windows
NBLK = 4         # projection t-blocks of 512
N_WARM = 20      # warmup matmuls (N=256) bridging the initial DMA wait

XOR16 = [i ^ 16 for i in range(32)]


# ----------------------------------------------------------------- device IR
def build_module(reps=1):
    nc = bacc.Bacc("TRN2", target_bir_lowering=False, debug=False,
                   num_devices=NCORES)

    xt = nc.dram_tensor("xt", [E, T], BF16, kind="ExternalInput").ap()
    wqkv = nc.dram_tensor("wqkv", [E, 3 * DL], BF16, kind="ExternalInput").ap()
    wot = nc.dram_tensor("wot", [DL, E], BF16, kind="ExternalInput").ap()
    trig = nc.dram_tensor("trig", [2, 128, T], BF16, kind="ExternalInput").ap()
    yt = nc.dram_tensor("yt", [E, T], BF16, kind="ExternalOutput").ap()

    with tile.TileContext(nc) as tc:
        for _ in range(reps):
            _body(tc, xt, wqkv, wot, trig, yt)
    nc.compile()
    return nc


def _body(tc, xt, wqkv, wot, trig, yt):
    nc = tc.nc

    with ExitStack() as ctx:
        po = ctx.enter_context(tc.tile_pool(name="po", bufs=1))
        xcp = ctx.enter_context(tc.tile_pool(name="xcp", bufs=8))
        rp = ctx.enter_context(tc.tile_pool(name="rp", bufs=6))
        ep = ctx.enter_context(tc.tile_pool(name="ep", bufs=5))
        dp = ctx.enter_context(tc.tile_pool(name="dp", bufs=6))
        yp = ctx.enter_context(tc.tile_pool(name="yp", bufs=2))
        pjp = ctx.enter_context(tc.tile_pool(name="pjp", bufs=2, space="PSUM"))
        ssp = ctx.enter_context(tc.tile_pool(name="ssp", bufs=2, space="PSUM"))
        sap = ctx.enter_context(tc.tile_pool(name="sap", bufs=2, space="PSUM"))

        # ---------------- persistent tiles --------------------------------
        # qk[0]=q pair0, qk[1]=q pair1, qk[2]=k pair0, qk[3]=k pair1
        qk = [po.tile([128, T], BF16, tag=f"qk{i}", name=f"qk{i}")
              for i in range(4)]
        # per (kt, local-head) slot of 128 lhsT columns: local head hh=2P+h
        # holds [v(64) | ones(64)] for h==0 and [ones(64) | v(64)] for h==1,
        # so the attnv matmul materializes the softmax denominator broadcast
        # across 64 psum rows for free (PE cost depends on moving dim only)
        v_sb = po.tile([128, 16 * 512], BF16, tag="v", name="v_sb")
        w_sb = po.tile([128, 8 * 768], BF16, tag="w", name="w_sb")
        wot_sb = [po.tile([128, E], BF16, tag=f"wot{p}", name=f"wot{p}")
                  for p in range(2)]
        trigc = po.tile([128, T], BF16, tag="tc", name="trigc")
        trigs = po.tile([128, T], BF16, tag="tsn", name="trigs")
        at = [po.tile([128, T], BF16, tag=f"at{p}", name=f"at{p}")
              for p in range(2)]
        warm = po.tile([128, 392], BF16, tag="warm", name="warm")

        # ---------------- init: memsets + DMAs -----------------------------
        nc.gpsimd.memset(warm[:], 0.0)
        # every (kt, head) slot is [ones(0:64) | v(64:128)]: the attnv matmul
        # then yields D on psum rows 0:64 and the numerator on rows 64:128
        v_ones = v_sb[:].rearrange("p (kt pp h j) -> p kt pp h j",
                                   kt=16, pp=2, h=2)
        nc.gpsimd.memset(v_ones[:, :, :, :, 0:64], 1.0)

        w_v = w_sb[:].rearrange("p (eo d) -> p eo d", eo=8)
        wqkv_v = wqkv.rearrange("(eo p) d -> p eo d", p=128)

        xc = {}

        def load_x(b, half):
            """one DMA for eo in [4*half, 4*half+4) of block b."""
            t_ = xcp.tile([128, 4 * 512], BF16, tag="xc", name="xc")
            tv = t_[:].rearrange("p (eo t) -> p eo t", eo=4)
            nc.sync.dma_start(
                out=tv,
                in_=xt[half * 512:(half + 1) * 512,
                       b * 512:(b + 1) * 512]
                .rearrange("(eo p) t -> p eo t", p=128))
            for i in range(4):
                xc[(b, 4 * half + i)] = tv[:, i, :]

        # issue order tuned so each transfer lands just before first use
        load_x(0, 0)
        nc.sync.dma_start(out=w_v[:, 0:4, 0:256], in_=wqkv_v[:, 0:4, 0:256])
        load_x(0, 1)
        nc.sync.dma_start(out=w_v[:, 4:8, 0:256], in_=wqkv_v[:, 4:8, 0:256])
        nc.sync.dma_start(out=w_v[:, :, 256:512], in_=wqkv_v[:, :, 256:512])
        nc.sync.dma_start(out=trigc[:, 0:512], in_=trig[0][:, 0:512])
        nc.sync.dma_start(out=trigs[:, 0:512], in_=trig[1][:, 0:512])
        nc.sync.dma_start(out=w_v[:, :, 512:768], in_=wqkv_v[:, :, 512:768])
        load_x(1, 0)
        load_x(1, 1)
        nc.sync.dma_start(out=trigc[:, 512:T], in_=trig[0][:, 512:T])
        nc.sync.dma_start(out=trigs[:, 512:T], in_=trig[1][:, 512:T])
        for p in range(2):
            nc.sync.dma_start(out=wot_sb[p][:],
                              in_=wot[p * 128:(p + 1) * 128, :])
        load_x(2, 0)
        load_x(2, 1)
        load_x(3, 0)
        load_x(3, 1)

        # activation-table load lands during the DMA wait (writes to a col
        # outside the warm matmul operand ranges so it doesn't gate them)
        nc.scalar.activation(warm[0:1, 384:385], warm[0:1, 0:1], AF.Exp)

        # warmup: keep PE busy (and ramping) until the first x chunks land
        warm_ps = pjp.tile([128, 256], F32, tag="pj", name="warm_ps")
        for i in range(N_WARM):
            nc.tensor.matmul(out=warm_ps[:], lhsT=warm[:, 0:128],
                             rhs=warm[:, 128:384],
                             start=(i == 0), stop=(i == N_WARM - 1))

        # ---------------- projection + rope emission helpers ---------------
        # nm: 0=q0, 1=q1, 2=k0, 3=k1 ; block b covers t cols [512b, 512b+512)
        def qk_mms(nm, b):
            """8 accumulating MMs + psum->bf16 copy; returns raw tile."""
            wcol = (nm % 2) * 128 if nm < 2 else 256 + (nm % 2) * 128
            ps = pjp.tile([128, 512], F32, tag="pj", name="pjqk")
            for eo in range(8):
                nc.tensor.matmul(
                    out=ps[:],
                    lhsT=w_sb[:, eo * 768 + wcol: eo * 768 + wcol + 128],
                    rhs=xc[(b, eo)][:],
                    start=(eo == 0), stop=(eo == 7))
            raw = rp.tile([128, 512], BF16, tag="raw", name="raw")
            if b == 0 and nm < 2:
                nc.scalar.copy(raw[:], ps[:])   # ACT is idle pre-attention
            else:
                nc.vector.tensor_copy(raw[:], ps[:])
            return raw

        def qk_rope(nm, b, raw):
            """row^16 partner swap (DVE shuffle) + cos/sin muls + add."""
            cs = slice(b * 512, b * 512 + 512)
            swp = rp.tile([128, 512], BF16, tag="swp", name="swp")
            nc.vector.stream_shuffle(swp[:], raw[:], XOR16)
            nc.gpsimd.tensor_mul(qk[nm][:, cs], raw[:], trigc[:, cs])
            tmp = rp.tile([128, 512], BF16, tag="tmp", name="tmp")
            nc.vector.tensor_mul(tmp[:], swp[:], trigs[:, cs])
            nc.vector.tensor_add(qk[nm][:, cs], qk[nm][:, cs], tmp[:])

        def v_grp(b, tt):
            """one 128-t-row V projection group; kt block = 4b+tt."""
            ps = pjp.tile([128, 256], F32, tag="pj", name="pjv")
            for eo in range(8):
                nc.tensor.matmul(
                    out=ps[:],
                    lhsT=xc[(b, eo)][:, tt * 128:tt * 128 + 128],
                    rhs=w_v[:, eo, 512:768],
                    start=(eo == 0), stop=(eo == 7))
            kt = 4 * b + tt
            slot4 = v_sb[:, kt * 512:(kt + 1) * 512] \
                .rearrange("p (pp h j) -> p pp h j", pp=2, h=2)
            ps4 = ps[:].rearrange("p (pp h x) -> p pp h x", pp=2, h=2)
            if b == 0 and tt % 2 == 0:
                nc.scalar.copy(slot4[:, :, :, 64:128], ps4[:])
            else:
                nc.vector.tensor_copy(slot4[:, :, :, 64:128], ps4[:])

        # ---------------- filler FIFO --------------------------------------
        fifo = deque()
        emitted = set()
        # rough PE-time of each item kind, for the debt-based pump
        COSTS = {"qkA": 1700.0, "qkB": 0.0, "v": 900.0, "yt": 480.0}
        debt = [0.0]

        def enqueue_block(b, b0_order=False):
            raws = {}
            if b0_order:
                # DMA arrival order at startup: A's first, then v/B
                # interleaved (psum-ring WARs hide behind alternation)
                order = [("A", 0), ("A", 2), ("A", 1), ("A", 3),
                         ("B", 0), ("B", 2), ("v", 0), ("B", 1),
                         ("v", 1), ("B", 3), ("v", 2), ("v", 3)]
            else:
                order = [("A", 0), ("A", 2), ("B", 0), ("A", 1), ("B", 2),
                         ("A", 3), ("B", 1), ("B", 3),
                         ("v", 0), ("v", 1), ("v", 2), ("v", 3)]
            for kind, x in order:
                if kind == "A":
                    fifo.append((("qkA", x, b),
                                 lambda nm=x, b=b: raws.__setitem__(
                                     nm, qk_mms(nm, b))))
                elif kind == "B":
                    fifo.append((("qkB", x, b),
                                 lambda nm=x, b=b: qk_rope(nm, b,
                                                           raws.pop(nm))))
                else:
                    fifo.append((("v", b, x),
                                 lambda b=b, tt=x: v_grp(b, tt)))

        def emit_next():
            tag, fn = fifo.popleft()
            fn()
            emitted.add(tag)

        reserve = [0]

        def pump_ns(ns):
            debt[0] += ns
            while len(fifo) > reserve[0] and debt[0] >= COSTS[fifo[0][0][0]]:
                k = fifo[0][0][0]
                emit_next()
                debt[0] -= COSTS[k]

        def force(tag):
            if tag in emitted:
                return
            while fifo:
                t, _ = fifo[0]
                emit_next()
                if t == tag:
                    debt[0] = 0.0
                    return
            raise AssertionError(f"force: {tag} never enqueued")

        # ---------------- normalization helpers -----------------------------
        IDENT32 = list(range(32))
        ZERO32 = [0] * 32

        def norm_h(P, W, h, c0, c1, ps_a, snapshot=False):
            """normalize cols [c0,c1) of pair P window W head-half h."""
            cw = c1 - c0
            sl = slice(c0, c1)
            gsl = slice(W * 512 + c0, W * 512 + c1)
            if snapshot:
                # stage the completed psum columns to SBUF with one short
                # copy so later attnv steps' writes only WAR against the
                # copy, not the whole norm chain
                stg = dp.tile([128, 512], F32, tag="stg", name="stg")
                if h == 0:
                    nc.scalar.copy(stg[:, 0:cw], ps_a[h][:, sl])
                else:
                    nc.vector.tensor_copy(stg[:, 0:cw], ps_a[h][:, sl])
                ps_a = {h: stg}
                sl = slice(0, cw)
            # D on psum rows 0:64, numerator on rows 64:128 for both heads.
            # reciprocal_approx_fast only works at partition base 0.
            rcb = dp.tile([128, 512], F32, tag="rcb", name="rcb")
            nc.vector.reciprocal_approx_fast(out=rcb[0:64, 0:cw],
                                             in_=ps_a[h][0:64, sl])
            nc.vector.stream_shuffle(rcb[64:128, 0:cw], rcb[0:64, 0:cw],
                                     IDENT32)
            if h == 1:
                nc.vector.tensor_mul(at[P][64:128, gsl], ps_a[1][64:128, sl],
                                     rcb[64:128, 0:cw])
            else:
                a0n = dp.tile([128, 512], BF16, tag="a0n", name="a0n")
                nc.vector.tensor_mul(a0n[64:128, 0:cw], ps_a[0][64:128, sl],
                                     rcb[64:128, 0:cw])
                nc.vector.stream_shuffle(at[P][0:64, gsl], a0n[64:128, 0:cw],
                                         IDENT32)

        # ---------------- output projection ---------------------------------
        ytv = yt.rearrange("(et p) t -> p et t", p=128)

        def enqueue_yt(W, c0=0, c1=512):
            qcols = slice(W * 512 + c0, W * 512 + c1)
            cw = c1 - c0
            y_sb = yp.tile([128, 8 * 512], BF16, tag="ysb", name="y_sb")
            tail_piece = cw < 512
            nst = 1 if tail_piece else 4   # store granularity (ets)

            def yt_grp(et):
                ps_y = pjp.tile([128, 512], F32, tag="pj", name="ps_y")
                for p in range(2):
                    nc.tensor.matmul(
                        out=ps_y[:, 0:cw],
                        lhsT=wot_sb[p][:, et * 128:(et + 1) * 128],
                        rhs=at[p][:, qcols],
                        start=(p == 0), stop=(p == 1))
                ydst = y_sb[:, et * 512:et * 512 + cw]
                if tail_piece and (c0 == 0 or et % 2 == 0):
                    # 3a: ACT takes all copies (DVE runs the norm chain);
                    # 3b: alternate so neither engine paces the PE
                    nc.scalar.copy(ydst, ps_y[:, 0:cw])
                else:
                    nc.vector.tensor_copy(ydst, ps_y[:, 0:cw])
                if tail_piece:
                    # batch stores: ets 0-3, 4-6, then et7 alone so the
                    # final DMA chain after the last copy is short
                    groups = {3: (0, 4), 6: (4, 3), 7: (7, 1)}
                    if et in groups:
                        e0, ne = groups[et]
                        src = (y_sb[:, e0 * 512:(e0 + ne) * 512]
                               .rearrange("p (et t) -> p et t", et=ne)[:, :, 0:cw]
                               if ne > 1 else ydst)
                        nc.sync.dma_start(out=ytv[:, e0:e0 + ne, qcols],
                                          in_=src)
                elif et % nst == nst - 1:
                    eg = et // nst
                    nc.sync.dma_start(
                        out=ytv[:, eg * nst:(eg + 1) * nst, qcols],
                        in_=y_sb[:, eg * nst * 512:(eg + 1) * nst * 512]
                        .rearrange("p (et t) -> p et t", et=nst))

            for et in range(8):
                fifo.append((("yt", W, et), lambda et=et: yt_grp(et)))

        # ---------------- attention ----------------------------------------
        def att_call(P, W, tail=False):
            """attention for pair P, q cols [512W, 512W+512)."""
            nkt = 4 * W + 4
            qcols = slice(W * 512, W * 512 + 512)
            # rope of q[P] block W and k[P] blocks <= W must be emitted
            force(("qkB", P, W))
            for bb in range(W + 1):
                force(("qkB", 2 + P, bb))

            ps_a = [sap.tile([128, 512], F32, tag="a", name="ps_a")
                    for _ in range(2)]
            exps = [None] * nkt

            def scores_step(kt):
                qs = max(0, 128 * kt - 512 * W)
                diag = kt >= 4 * W
                ss = ssp.tile([128, 1024], F32, tag="s", name="ss")
                for h in range(2):
                    nc.tensor.matmul(
                        out=ss[:, h * 512 + qs: h * 512 + 512],
                        lhsT=qk[2 + P][h * 64:h * 64 + 64,
                                       kt * 128:kt * 128 + 128],
                        rhs=qk[P][h * 64:h * 64 + 64, W * 512 + qs:
                                  W * 512 + 512],
                        start=True, stop=True,
                        tile_position=(h * 64, 0))
                e = ep.tile([128, 1024], BF16, tag="e", name="exp_t")
                e3 = e[:].rearrange("p (h c) -> p h c", h=2)[:, :, qs:512]
                s3 = ss[:].rearrange("p (h c) -> p h c", h=2)[:, :, qs:512]
                nc.scalar.activation(e3, s3, AF.Exp)
                if diag:
                    ed = e[:].rearrange("p (h c) -> p h c",
                                        h=2)[:, :, qs:qs + 128]
                    nc.gpsimd.affine_select(
                        ed, ed, [[0, 2], [1, 128]], ALU.is_ge, 0.0,
                        base=0, channel_multiplier=-1)
                exps[kt] = (e, qs)

            def attnv_step(kt):
                e, qs = exps[kt]
                for h in range(2):
                    slot = kt * 512 + (2 * P + h) * 128
                    nc.tensor.matmul(
                        out=ps_a[h][0:128, qs:512],
                        lhsT=v_sb[:, slot:slot + 128],
                        rhs=e[:, h * 512 + qs: h * 512 + 512],
                        start=(kt == 0), stop=(kt == nkt - 1))
                exps[kt] = None

            for step in range(nkt + 1):
                if step < nkt:
                    # pre-force v blocks one block ahead of the kt cursor
                    vb = min(step // 4 + 1, W)
                    for bb in range(vb + 1):
                        for tt in range(4):
                            if (("v", bb, tt)) not in emitted:
                                force(("v", bb, tt))
                    scores_step(step)
                if step > 0:
                    attnv_step(step - 1)
                if tail and step == nkt - 2:
                    # attnv of kt=nkt-3 is done: window cols 0:256 complete;
                    # normalize them and make their yt groups available so
                    # the PE has real work through the final steps
                    norm_h(P, W, 1, 0, 256, ps_a)
                    norm_h(P, W, 0, 0, 256, ps_a)
                    enqueue_yt(W, 0, 256)
                    reserve[0] = 0   # norm latency window: release holdbacks
                # ACT-vs-PE imbalance this step, paid to the filler pump
                qs = max(0, 128 * min(step, nkt - 1) - 512 * W)
                cols = 512 - qs
                gap = (2 * cols * 0.833 + 500.0) - (4 * cols * 0.4167 + 107.0)
                pump_ns(max(200.0, gap))

            # ---------------- normalization -------------------------------
            # h1 first: its cross-partition shuffle is the longest pole
            if tail:
                norm_h(P, W, 1, 256, 512, ps_a)
                norm_h(P, W, 0, 256, 512, ps_a)
                enqueue_yt(W, 256, 512)
            else:
                pump_ns(600.0)
                norm_h(P, W, 1, 0, 512, ps_a)
                pump_ns(600.0)
                norm_h(P, W, 0, 0, 512, ps_a)
                pump_ns(600.0)

        # ---------------- master schedule -----------------------------------
        # block 0 emitted straight; blocks 1..3 via the FIFO
        enqueue_block(0, b0_order=True)
        while fifo:
            emit_next()
        for b in range(1, NBLK):
            enqueue_block(b)

        RESV = {(0, 2): 8, (1, 2): 8, (0, 3): 14, (1, 3): 8}
        for W in range(NWIN):
            for P in range(2):
                reserve[0] = RESV.get((P, W), 0)
                att_call(P, W, tail=(P == 1 and W == NWIN - 1))
            if W < NWIN - 1:
                enqueue_yt(W)
        while fifo:
            emit_next()


# ----------------------------------------------------------------- host side
def _prep_core_inputs(x, wq, wk, wv, wo):
    """Build the 8 per-core input dicts."""
    import ml_dtypes
    bf = ml_dtypes.bfloat16

    inv_freq = 1.0 / (ROPE_BASE ** (np.arange(0, HD, 2, dtype=np.float32) / HD))
    pos = np.arange(T, dtype=np.float32)
    freqs = pos[:, None] * inv_freq[None, :]          # [T, 32]
    cosT = np.cos(freqs).T.astype(np.float32)          # [32, T]
    sinT = np.sin(freqs).T.astype(np.float32)
    # per-head 64-row layout [e0:16 | o0:16 | e16:32 | o16:32] (partner = r^16)
    C64 = np.concatenate([cosT[0:16], cosT[0:16], cosT[16:32], cosT[16:32]])
    S64 = np.concatenate([-sinT[0:16], sinT[0:16], -sinT[16:32], sinT[16:32]])
    C = np.tile(C64, (2, 1))                           # [128, T]
    S = np.tile(S64, (2, 1))
    trig = np.stack([C, S]).astype(bf)                 # [2, 128, T]
    scale = np.float32(1.0 / np.sqrt(HD))              # folded into wq

    evens = np.arange(0, HD, 2)
    odds = np.arange(1, HD, 2)
    perm64 = np.concatenate([evens[0:16], odds[0:16], evens[16:32],
                             odds[16:32]])

    xts = [np.ascontiguousarray(x[b_].T).astype(bf) for b_ in range(B)]
    in_maps = []
    for core in range(NCORES):
        b_, hg = divmod(core, 4)
        heads = np.arange(4 * hg, 4 * hg + 4)
        qk_rows = np.concatenate([h * HD + perm64 for h in heads])
        v_rows = np.concatenate([h * HD + np.arange(HD) for h in heads])
        wq_t = wq.T[:, qk_rows] * scale                # [E, 256]
        wk_t = wk.T[:, qk_rows]
        wv_t = wv.T[:, v_rows]
        wqkv = np.concatenate([wq_t, wk_t, wv_t], axis=1).astype(bf)
        wot_ = np.ascontiguousarray(wo.T[v_rows, :]).astype(bf)
        in_maps.append({
            "xt": xts[b_], "wqkv": wqkv, "wot": wot_,
            "trig": trig,
        })
    return in_maps


_NC_CACHE = {}


def _get_module():
    if "nc" not in _NC_CACHE:
        _NC_CACHE["nc"] = build_module()
    return _NC_CACHE["nc"]


def _get_runner(key="nc", builder=None):
    """Build (once) a cached jax.jit shard_map callable over the 8 cores."""
    rkey = "runner_" + key
    if rkey in _NC_CACHE:
        return _NC_CACHE[rkey]
    import jax
    import concourse.mybir as _mb
    from concourse import bass2jax as b2j
    from jax.sharding import Mesh, PartitionSpec
    from jax.experimental.shard_map import shard_map

    if key == "nc":
        nc = _get_module()
    else:
        if key not in _NC_CACHE:
            _NC_CACHE[key] = builder()
        nc = _NC_CACHE[key]
    b2j.install_neuronx_cc_hook()
    partition_name = (nc.partition_id_tensor.name
                      if nc.partition_id_tensor else None)
    in_names, out_names, out_avals, zero_outs = [], [], [], []
    for alloc in nc.m.functions[0].allocations:
        if not isinstance(alloc, _mb.MemoryLocationSet):
            continue
        name = alloc.memorylocations[0].name
        if alloc.kind == "ExternalInput":
            if name != partition_name:
                in_names.append(name)
        elif alloc.kind == "ExternalOutput":
            out_names.append(name)
            shape = tuple(alloc.tensor_shape)
            dtype = _mb.dt.np(alloc.dtype)
            out_avals.append(jax.core.ShapedArray(shape, dtype))
            zero_outs.append(np.zeros(shape, dtype))
    n_params = len(in_names)
    all_names = list(in_names) + list(out_names)
    if partition_name is not None:
        all_names.append(partition_name)

    def _body_fn(*args):
        operands = list(args)
        if partition_name is not None:
            operands.append(b2j.partition_id_tensor())
        outs = b2j._bass_exec_p.bind(
            *operands,
            out_avals=tuple(out_avals),
            in_names=tuple(all_names),
            out_names=tuple(out_names),
            lowering_input_output_aliases=(),
            sim_require_finite=True,
            sim_require_nnan=True,
            nc=nc,
        )
        return tuple(outs)

    devices = jax.devices()[:NCORES]
    mesh = Mesh(np.asarray(devices), ("core",))
    n_outs = len(out_names)
    in_specs = (PartitionSpec("core"),) * (n_params + n_outs)
    out_specs = (PartitionSpec("core"),) * n_outs
    sharded = jax.jit(
        shard_map(_body_fn, mesh=mesh, in_specs=in_specs,
                  out_specs=out_specs, check_rep=False),
        keep_unused=True)
    from jax.sharding import NamedSharding
    _shard = NamedSharding(mesh, PartitionSpec("core"))
    concat_zeros = [
        jax.device_put(
            np.zeros((NCORES * z.shape[0], *z.shape[1:]), z.dtype), _shard)
        for z in zero_outs
    ]
    runner = {
        "sharded": sharded, "in_names": in_names, "out_names": out_names,
        "out_avals": out_avals, "concat_zeros": concat_zeros,
    }
    _NC_CACHE[rkey] = runner
    return runner


_CONST_NAMES = {"trig"}


def _run_spmd_cached(in_maps):
    import jax
    r = _get_runner()
    ckey = "const_dev"
    if ckey not in _NC_CACHE:
        _NC_CACHE[ckey] = {}
    const_dev = _NC_CACHE[ckey]
    concat_in = []
    for nm in r["in_names"]:
        if nm in _CONST_NAMES:
            if nm not in const_dev:
                arr = np.concatenate(
                    [np.asarray(in_maps[c][nm]) for c in range(NCORES)],
                    axis=0)
                const_dev[nm] = jax.device_put(arr)
            concat_in.append(const_dev[nm])
        else:
            concat_in.append(np.concatenate(
                [np.asarray(in_maps[c][nm]) for c in range(NCORES)], axis=0))
    out_arrs = r["sharded"](*concat_in, *r["concat_zeros"])
    nm = r["out_names"]
    av = r["out_avals"]
    return [
        {nm[i]: np.asarray(out_arrs[i]).reshape(NCORES, *av[i].shape)[c]
         for i in range(len(nm))}
        for c in range(NCORES)
    ]


def kernel(x, wq, wk, wv, wo, _trace=False, _trace_kwargs=None):
    x = np.asarray(x, dtype=np.float32)
    wq = np.asarray(wq, dtype=np.float32)
    wk = np.asarray(wk, dtype=np.float32)
    wv = np.asarray(wv, dtype=np.float32)
    wo = np.asarray(wo, dtype=np.float32)

    in_maps = _prep_core_inputs(x, wq, wk, wv, wo)
    try:
        results = _run_spmd_cached(in_maps)
    except Exception:
        nc = _get_module()
        results = run_bass_kernel_spmd(
            nc, in_maps, core_ids=list(range(NCORES))).results
    out = np.empty((B, T, E), dtype=np.float32)
    for b_ in range(B):
        acc = np.zeros((E, T), dtype=np.float32)
        for g in range(4):
            acc += results[4 * b_ + g]["yt"].astype(np.float32)
        out[b_] = acc.T
    return out


if __name__ == "__main__":
    nc = _get_module()
    print("module built ok")
